# revision 1
# baseline (speedup 1.0000x reference)
"""Trainium2 Bass kernel for nn_Attention_30468497997979.

Reference computation (per batch b of 8):
    X = hidden_states[b,:,0,:]              # (C=768, S=384)
    Q/K/V = W @ X + b                       # 1x1 conv == channel matmul
    per head h (12 heads, head dim 64, channel c = d*12 + h):
        scores = (Q_h^T K_h) / 8, mask q>k, softmax over k
        attn_h = V_h @ softmax
    out = Wo @ concat_heads(attn)           # channel c = h*64 + d

Sharding: pure data-parallel, one batch per NeuronCore (8 cores).

Per-core kernel layout choices:
  - Host pre-permutes W_{q,k,v} rows to head-major channel order
    (c' = h*64 + d) and transposes all weights to [c_in, c_out] so the
    contraction dim lands on SBUF partitions. 1/sqrt(d) folded into Wq/bq.
  - scores are computed transposed ([k, q] with keys on partitions):
    lhsT = K_h k-chunk, rhs = Q_h. Softmax needs no max-subtraction
    (scores are O(1); masked entries get -1e4 -> exp == 0).
  - V is projected directly in transposed [s, c'] layout (lhsT = X chunk,
    rhs = WvT), so the attn@V matmul contracts over k on partitions with
    no on-chip transposes anywhere.
  - The softmax denominator is fused into the attn@V matmul as an extra
    ones-column appended to each V tile (psum row 64 = column sums).
  - The V bias commutes through attention exactly (softmax rows sum to 1)
    and is folded on the host into an output-projection bias Wo @ bv.
  - Normalization is deferred: denominator rows collect in SBUF, 1/sums
    runs as a few batched DVE reciprocals, and each row is broadcast
    across partitions with a K=1 PE matmul before a DVE multiply.
  - Matmul data is bf16 (the PE streams 1 output row/cycle vs ~2 for
    fp32r; half the HBM traffic). PSUM accumulation stays fp32; measured
    ~4e-3 relative error vs the fp32 reference. Set QK_DT/V_DT/O_DT to
    "f32r" for a ~2.7e-4-error, ~16%-slower variant.
"""

import numpy as np

B, C, S, H, D = 8, 768, 384, 12, 64
NC_CHUNKS = C // 128  # 6
NEG = -10000.0

# matmul dtypes per stage: "f32r" (full fp32 data, ~2 PE cycles/row) or
# "bf16" (1 cycle/row, half the DMA bytes, ~1e-3 rel err)
QK_DT = "bf16"   # x, Wq, Wk, q, k (score path)
V_DT = "bf16"    # x2, Wv, vt, e (attn@V path)
O_DT = "bf16"    # Wo, attn (output projection)

_STATE = {}


# --------------------------------------------------------------------------
# Workaround: this walrus build rejects the multi-wait InstDrain that
# TileContext emits at exit ("Too many sync wait commands"). Split the
# drain's sem waits onto standalone sync-engine wait instructions.
def _patch_walrus_ldw_opt():
    """Enable walrus's load-weight pipelining (ldw-opt): overlaps each
    matmul's LDWEIGHTS with the previous matmul's execution."""
    import os
    import concourse.bass_utils as bu

    if os.environ.get("KERNEL_LDW_OPT") != "1":
        return
    if getattr(bu, "_ldw_opt_patch", False):
        return
    orig = bu.run_command

    def patched(argv, **kwargs):
        argv = [
            a.replace("--enable-ldw-opt=false", "--enable-ldw-opt=true")
            if isinstance(a, str)
            else a
            for a in argv
        ]
        return orig(argv, **kwargs)

    bu.run_command = patched
    bu._ldw_opt_patch = True


def _patch_tile_drain():
    import concourse.tile as tile_mod
    from concourse.vector_clock import ScopedClock
    from bass_rust import SyncInfo

    if getattr(tile_mod.TileContext, "_drain_split_patch", False):
        return

    def _drain_and_barrier_split(self, tick_clock, wait_clock):
        nc = self.nc
        assert self.sems is not None
        handles = {}
        for h in self.sems.allocated().values():
            handles[h.num] = h
            handles[h.name] = h

        probe = nc.sync.nop()
        wait_clock.add_sem_waits(
            probe.ins, ScopedClock({None: tick_clock.global_clock})
        )
        waits = list(probe.ins.sync_info.on_wait)
        probe.ins.sync_info = SyncInfo(on_wait=[], on_update=[])
        for w in waits:
            h = handles.get(w.id) or handles.get(w.ant_name)
            if h is not None:
                nc.sync.wait_ge(h, w.wait_value)
            else:
                n2 = nc.sync.nop()
                n2.ins.sync_info = SyncInfo(on_wait=[w], on_update=[])

        drain_inst = nc.sync.drain()
        wait_clock.add_sem_waits(
            drain_inst.ins, ScopedClock({None: tick_clock.global_clock})
        )
        if list(drain_inst.ins.sync_info.on_wait):
            drain_inst.ins.sync_info = SyncInfo(on_wait=[], on_update=[])

        nc.all_engine_barrier()
        popped = nc._tile_sem_poison_stack.pop()
        assert popped is self._sem_poison
        nc.clear_and_free_semaphores(list(self.sems.allocated().values()))
        nc.all_engine_barrier()

        # This walrus codegen supports at most ONE sem wait per
        # instruction. Move extra waits onto same-engine nop carriers
        # inserted just before the instruction (engine queues execute in
        # order, so the semantics are identical).
        import concourse.mybir as mybir

        k = 0
        for f in nc.m.functions:
            for bb in f.blocks:
                new_insts = []
                for inst in bb.instructions:
                    si = inst.sync_info
                    waits = list(si.on_wait) if si else []
                    if len(waits) > 1:
                        for w in waits[:-1]:
                            nop = mybir.InstNoOp(name=f"I-wsplit-{k}")
                            k += 1
                            nop.engine = inst.engine
                            nop.sync_info = SyncInfo(on_wait=[w], on_update=[])
                            nc.register_instruction(nop)
                            new_insts.append(nop)
                        inst.sync_info = SyncInfo(
                            on_wait=[waits[-1]], on_update=list(si.on_update)
                        )
                    new_insts.append(inst)
                bb.instructions = new_insts

    tile_mod.TileContext._drain_and_barrier = _drain_and_barrier_split
    tile_mod.TileContext._drain_split_patch = True


# --------------------------------------------------------------------------
def _build_nc(use_f32r=True):
    import concourse.bass as bass
    import concourse.mybir as mybir
    import concourse.tile as tile

    _patch_tile_drain()
    _patch_walrus_ldw_opt()

    f32 = mybir.dt.float32
    f32r = mybir.dt.float32r
    bf16 = mybir.dt.bfloat16
    Ident = mybir.ActivationFunctionType.Identity
    Exp = mybir.ActivationFunctionType.Exp

    dmap = {"f32r": f32r, "bf16": bf16, "f32": f32}
    dt_qk, dt_v, dt_o = dmap[QK_DT], dmap[V_DT], dmap[O_DT]
    dtm = f32r if use_f32r else f32

    nc = bass.Bass()
    x_d = nc.dram_tensor("x", [C, S], dt_qk, kind="ExternalInput")
    x2_d = (
        nc.dram_tensor("x2", [C, S], dt_v, kind="ExternalInput")
        if dt_v != dt_qk
        else None
    )
    wq_d = nc.dram_tensor("wqt", [C, C], dt_qk, kind="ExternalInput")
    wk_d = nc.dram_tensor("wkt", [C, C], dt_qk, kind="ExternalInput")
    wv_d = nc.dram_tensor("wvt", [C, C], dt_v, kind="ExternalInput")
    wo_d = nc.dram_tensor("wot", [C, C], dt_o, kind="ExternalInput")
    bq_d = nc.dram_tensor("bq", [C, 1], f32, kind="ExternalInput")
    bk_d = nc.dram_tensor("bk", [C, 1], f32, kind="ExternalInput")
    # V-bias folded through attention (softmax rows sum to 1) into a
    # host-precomputed output bias: obias = Wo @ bv_headmajor
    ob_d = nc.dram_tensor("obias", [C, 1], f32, kind="ExternalInput")
    # diagonal 128x128 triangle blocks of the [k, q] mask, stacked
    mask_d = nc.dram_tensor("maskd", [S, 128], f32, kind="ExternalInput")
    konst_d = nc.dram_tensor("konst", [128, D], dtm, kind="ExternalInput")
    konstv_d = (
        nc.dram_tensor("konstv", [128, D], dt_v, kind="ExternalInput")
        if dt_v != dtm
        else None
    )
    y_d = nc.dram_tensor("y", [C, S], f32, kind="ExternalOutput")

    with tile.TileContext(nc) as tc:
        with (
            tc.tile_pool(name="persist", bufs=1) as persist,
            tc.tile_pool(name="epool", bufs=9) as epool,
            tc.tile_pool(name="small", bufs=4) as small,
            tc.tile_pool(name="psA", bufs=2, space="PSUM") as psA,
            tc.tile_pool(name="psS", bufs=4, space="PSUM") as psS,
            tc.tile_pool(name="psV", bufs=1, space="PSUM") as psV,
            tc.tile_pool(name="psR", bufs=1, space="PSUM") as psR,
        ):
            # ---- loads -------------------------------------------------
            # x and wv chunks alternate across the Sync and GpSimd queues
            # (first compute needs x0+wv0 ASAP); wq/wo load as single big
            # DMAs on the Activation queue (idle at start), wk on GpSimd.
            xt = [
                persist.tile([128, S], dt_qk, tag=f"x{i}", name=f"x{i}")
                for i in range(NC_CHUNKS)
            ]
            wv_sb = [
                persist.tile([128, C], dt_v, tag=f"wv{i}", name=f"wv{i}")
                for i in range(NC_CHUNKS)
            ]
            if x2_d is not None:
                xv = [
                    persist.tile([128, S], dt_v, tag=f"xv{i}", name=f"xv{i}")
                    for i in range(NC_CHUNKS)
                ]
            else:
                xv = xt
            for i in range(NC_CHUNKS):
                ex, ev = (nc.sync, nc.gpsimd) if i % 2 == 0 else (nc.gpsimd, nc.sync)
                ev.dma_start(wv_sb[i][:], wv_d[i * 128 : (i + 1) * 128, :])
                if x2_d is not None:
                    ex.dma_start(xv[i][:], x2_d[i * 128 : (i + 1) * 128, :])
                ex.dma_start(xt[i][:], x_d[i * 128 : (i + 1) * 128, :])

            def load_w_mono(dram, tag, eng, dt_):
                t = persist.tile([128, NC_CHUNKS, C], dt_, tag=tag, name=tag)
                eng.dma_start(
                    t[:], dram.rearrange("(cc p) c -> p cc c", p=128)
                )
                return [t[:, i, :] for i in range(NC_CHUNKS)]

            wq_sb = load_w_mono(wq_d, "wq", nc.scalar, dt_qk)
            wk_sb = load_w_mono(wk_d, "wk", nc.gpsimd, dt_qk)
            wo_sb = load_w_mono(wo_d, "wo", nc.scalar, dt_o)

            def load_b(dram, tag):
                tiles = []
                for i in range(NC_CHUNKS):
                    t = persist.tile([128, 1], f32, tag=f"{tag}{i}", name=f"{tag}{i}")
                    nc.gpsimd.dma_start(t[:], dram[i * 128 : (i + 1) * 128, :])
                    tiles.append(t)
                return tiles

            bq_sb = load_b(bq_d, "bq")
            bk_sb = load_b(bk_d, "bk")
            ob_sb = load_b(ob_d, "ob")

            mask_sb = []
            for kc in range(3):
                t = persist.tile([128, 128], f32, tag=f"mask{kc}", name=f"mask{kc}")
                nc.gpsimd.dma_start(t[:], mask_d[kc * 128 : (kc + 1) * 128, :])
                mask_sb.append(t)

            # ---- V projection, transposed: vt[sq][s, h, 0:64] = V'[c', s]^T
            # col 64 of each head slot = 1.0 (fused denominator column)
            vt = []
            for sq in range(3):
                t = persist.tile([128, H, D + 1], dt_v, tag=f"vt{sq}", name=f"vt{sq}")
                kd = konstv_d if konstv_d is not None else konst_d
                nc.gpsimd.dma_start(
                    t[:, :, D : D + 1],
                    kd[:, 0:H].rearrange("p (h o) -> p h o", o=1),
                )
                vt.append(t)
            for sq in range(3):
                for half in range(2):
                    ps = psA.tile([128, S], f32, tag="proj", name="proj")
                    for cc in range(NC_CHUNKS):
                        nc.tensor.matmul(
                            ps[:],
                            xv[cc][:, sq * 128 : (sq + 1) * 128],
                            wv_sb[cc][:, half * 384 : (half + 1) * 384],
                            start=(cc == 0),
                            stop=(cc == NC_CHUNKS - 1),
                        )
                    nc.vector.tensor_copy(
                        vt[sq][:, half * 6 : (half + 1) * 6, 0:D],
                        ps[:].rearrange("p (h d) -> p h d", d=D),
                    )

            # ---- Q/K projections (head-major rows; scale folded into Wq)
            q_sb = [
                persist.tile([128, S], dt_qk, tag=f"q{oc}", name=f"q{oc}")
                for oc in range(NC_CHUNKS)
            ]
            k_sb = [
                persist.tile([128, S], dt_qk, tag=f"k{oc}", name=f"k{oc}")
                for oc in range(NC_CHUNKS)
            ]

            def proj(oc, w_tiles, bias, out):
                ps = psA.tile([128, S], f32, tag="proj", name="proj")
                for cc in range(NC_CHUNKS):
                    nc.tensor.matmul(
                        ps[:],
                        w_tiles[cc][:, oc * 128 : (oc + 1) * 128],
                        xt[cc],
                        start=(cc == 0),
                        stop=(cc == NC_CHUNKS - 1),
                    )
                nc.scalar.activation(out[:], ps[:], Ident, bias=bias[:])

            # ---- attention -------------------------------------------
            # per head: scores^T -> masked exp -> attn@V with fused
            # denominator row. Normalization deferred: unnormalized attn
            # and the denominator rows land in SBUF; reciprocals run
            # batched per 6-head group, broadcast via one strided DMA.
            attn_un = [
                persist.tile([128, S], f32, tag=f"au{oc}", name=f"au{oc}")
                for oc in range(NC_CHUNKS)
            ]
            attn_sb = [
                persist.tile([128, S], dt_o, tag=f"at{oc}", name=f"at{oc}")
                for oc in range(NC_CHUNKS)
            ]
            sums_sb = persist.tile([H, S], f32, tag="sums", name="sums")
            nc.vector.memset(sums_sb[:], 1.0)
            rinv_sb = persist.tile([H, S], f32, tag="rinv", name="rinv")
            rinv_r = persist.tile([1, H, S], dtm, tag="rinvr", name="rinvr")
            ones_sb = persist.tile([1, D], dtm, tag="ones", name="ones")
            nc.gpsimd.dma_start(ones_sb[:], konst_d[0:1, :])

            def head_scores(h):
                oc, prow = h // 2, (h % 2) * D
                Qh = q_sb[oc][prow : prow + D, :]
                Kh = k_sb[oc][prow : prow + D, :]
                e_tiles = []
                for kc in range(3):
                    ncols = 128 * (kc + 1)
                    ps_s = psS.tile([128, S], f32, tag="sc", name="sc")
                    nc.tensor.matmul(
                        ps_s[:, 0:ncols],
                        Kh[:, kc * 128 : (kc + 1) * 128],
                        Qh[:, 0:ncols],
                        start=True,
                        stop=True,
                    )
                    # causal mask: only the diagonal block needs masking
                    nc.vector.tensor_add(
                        ps_s[:, kc * 128 : ncols],
                        ps_s[:, kc * 128 : ncols],
                        mask_sb[kc][:],
                    )
                    e = epool.tile([128, S], dt_v, tag="e", name="e")
                    nc.scalar.activation(e[:, 0:ncols], ps_s[:, 0:ncols], Exp)
                    e_tiles.append(e)
                return e_tiles

            def head_av(h, e_tiles):
                # attn@V; accumulate widest first so every element's first
                # write carries the start flag
                oc, prow = h // 2, (h % 2) * D
                ps_av = psV.tile([D + 1, S], f32, tag="av", name="av")
                for step, kc in enumerate([2, 1, 0]):
                    ncols = 128 * (kc + 1)
                    nc.tensor.matmul(
                        ps_av[:, 0:ncols],
                        vt[kc][:, h, :],
                        e_tiles[kc][:, 0:ncols],
                        start=(step == 0),
                        stop=(step == 2),
                        skip_group_check=True,
                    )
                st = small.tile([1, S], f32, tag="st", name="st")
                nc.vector.tensor_copy(st[:], ps_av[D : D + 1, :])
                nc.gpsimd.dma_start(sums_sb[h : h + 1, :], st[:])
                nc.scalar.copy(attn_un[oc][prow : prow + D, :], ps_av[0:D, :])

            def normalize_group(h0, h1):
                # rinv = 1/sums for heads [h0,h1) — one fused-NR DVE op on
                # h1-h0 lanes; a tiny DMA repacks the rows into one
                # partition's free dim (f32r view) so a K=1 matmul can
                # broadcast each row across 64 partitions.
                sl = slice(h0, h1)
                # compute engines need 32-aligned partition starts: run the
                # reciprocal over all 12 rows from partition 0 (later rows
                # recompute to the same values; unready rows are never read)
                nc.vector.reciprocal(rinv_sb[:, :], sums_sb[:, :])
                # repack rows into one partition (f32r view), then K=1
                # matmuls broadcast each row across 64 partitions
                nc.sync.dma_start(rinv_r[0:1, sl, :], rinv_sb[sl, :].bitcast(dtm))
                for h in range(h0, h1):
                    oc, prow = h // 2, (h % 2) * D
                    ps_r = psR.tile([D, S], f32, tag="rb", name="rb")
                    nc.tensor.matmul(
                        ps_r[:], ones_sb[:], rinv_r[0:1, h, :],
                        start=True, stop=True,
                    )
                    nc.vector.tensor_mul(
                        attn_sb[oc][prow : prow + D, :],
                        attn_un[oc][prow : prow + D, :],
                        ps_r[:],
                    )

            # two-stage software pipeline across heads: head h+1's score
            # matmuls are queued before head h's attn@V, so the PE has real
            # work during the mask->exp latency instead of a static stall
            pending = None
            for oc in range(NC_CHUNKS):
                proj(oc, wq_sb, bq_sb[oc], q_sb[oc])
                proj(oc, wk_sb, bk_sb[oc], k_sb[oc])
                for h in (2 * oc, 2 * oc + 1):
                    e_tiles = head_scores(h)
                    if pending is not None:
                        head_av(*pending)
                    pending = (h, e_tiles)
                    if h - 1 == 5:
                        normalize_group(0, 6)
                    if h - 1 == 9:
                        normalize_group(6, 10)
            head_av(*pending)
            normalize_group(10, 12)

            # ---- output projection (bias = host-folded Wo @ bv) --------
            for oc in range(NC_CHUNKS):
                ps = psA.tile([128, S], f32, tag="proj", name="proj")
                for cc in range(NC_CHUNKS):
                    nc.tensor.matmul(
                        ps[:],
                        wo_sb[cc][:, oc * 128 : (oc + 1) * 128],
                        attn_sb[cc],
                        start=(cc == 0),
                        stop=(cc == NC_CHUNKS - 1),
                    )
                ot = epool.tile([128, S], f32, tag="o", name="o", bufs=3)
                nc.scalar.activation(ot[:], ps[:], Ident, bias=ob_sb[oc][:])
                nc.sync.dma_start(y_d[oc * 128 : (oc + 1) * 128, :], ot[:])

    return nc


def _get_nc():
    if "nc" not in _STATE:
        _STATE["nc"] = _build_nc()
    return _STATE["nc"]


# --------------------------------------------------------------------------
def _np_dt(name):
    if name == "bf16":
        import ml_dtypes

        return ml_dtypes.bfloat16
    return np.float32


def _prep_maps(inputs):
    hs = np.asarray(inputs["hidden_states"], dtype=np.float32)
    Wq = np.asarray(inputs["Wq"], dtype=np.float32)
    bq = np.asarray(inputs["bq"], dtype=np.float32)
    Wk = np.asarray(inputs["Wk"], dtype=np.float32)
    bk = np.asarray(inputs["bk"], dtype=np.float32)
    Wv = np.asarray(inputs["Wv"], dtype=np.float32)
    bv = np.asarray(inputs["bv"], dtype=np.float32)
    Wo = np.asarray(inputs["Wo"], dtype=np.float32)

    # head-major channel permutation: c' = h*64 + d  <-  c = d*12 + h
    idx = (np.arange(H)[:, None] + np.arange(D)[None, :] * H).reshape(C)
    scale = float(D) ** -0.5

    dqk, dv, do = _np_dt(QK_DT), _np_dt(V_DT), _np_dt(O_DT)
    wqt = np.ascontiguousarray((scale * Wq[idx, :]).T).astype(dqk)
    wkt = np.ascontiguousarray(Wk[idx, :].T).astype(dqk)
    wvt = np.ascontiguousarray(Wv[idx, :].T).astype(dv)
    wot = np.ascontiguousarray(Wo.T).astype(do)
    bq2 = np.ascontiguousarray((scale * bq[idx]).reshape(C, 1))
    bk2 = np.ascontiguousarray(bk[idx].reshape(C, 1))

    # mask[k, q] = NEG where k < q; per k-chunk only the diagonal
    # triangle block needs masking
    blk = np.triu(np.full((128, 128), NEG, dtype=np.float32), 1)
    maskd = np.ascontiguousarray(np.tile(blk, (3, 1)))
    konst = np.ones((128, D), dtype=np.float32)
    # V-bias folded through attention (softmax rows sum to 1):
    # attn' = attn_nobias' + bv[idx], so out += Wo @ bv[idx]
    obias = np.ascontiguousarray((Wo @ bv[idx]).reshape(C, 1).astype(np.float32))

    shared = {
        "wqt": wqt, "wkt": wkt, "wvt": wvt, "wot": wot,
        "bq": bq2, "bk": bk2, "obias": obias, "maskd": maskd,
        "konst": konst,
    }
    if V_DT != "f32r":
        shared["konstv"] = konst.astype(dv)
    maps = []
    for b in range(B):
        xb = np.ascontiguousarray(hs[b, :, 0, :])
        m = {"x": xb.astype(dqk), **shared}
        if V_DT != QK_DT:
            m["x2"] = xb.astype(dv)
        maps.append(m)
    return maps


def _run(inputs, trace=False, **kwargs):
    from concourse.bass_utils import run_bass_kernel_spmd

    nc = _get_nc()
    in_maps = _prep_maps(inputs)
    res = run_bass_kernel_spmd(
        nc, in_maps, core_ids=list(range(B)), trace=trace, **kwargs
    )
    out = np.stack([res.results[b]["y"] for b in range(B)], axis=0)
    return out.reshape(B, C, 1, S).astype(np.float32), res


def kernel(**inputs):
    out, _ = _run(inputs, trace=False)
    return out



# revision 10
# speedup vs baseline: 1.2296x; 1.2296x over previous
"""Trainium2 Bass kernel for nn_Attention_30468497997979.

Reference computation (per batch b of 8):
    X = hidden_states[b,:,0,:]              # (C=768, S=384)
    Q/K/V = W @ X + b                       # 1x1 conv == channel matmul
    per head h (12 heads, head dim 64, channel c = d*12 + h):
        scores = (Q_h^T K_h) / 8, mask q>k, softmax over k
        attn_h = V_h @ softmax
    out = Wo @ concat_heads(attn)           # channel c = h*64 + d
Sharding: pure data-parallel, one batch per NeuronCore (8 cores).

Per-core kernel design (v2 — DMA/schedule-optimized):
  - All matmul data is bf16 (1 PE col/cycle); PSUM accumulation fp32.
  - Host pre-permutes W_{q,k,v} rows to head-major channel order
    (c' = h*64 + d), transposes to [c_in, c_out], folds 1/sqrt(d) into
    Wq/bq, folds the V bias through attention (softmax rows sum to 1)
    into an output bias Wo @ bv.
  - Weights are host-packed so every SBUF tile is one contiguous
    hardware-DMA: wq/wk/wo as six per-output-block [128, 6*128] loads
    (the kernel consumes output blocks one at a time, so each block
    gets its own completion semaphore), wv/x as [p][chunk][col] packs.
  - DMA priority: x split across three queues first, then wq0/wk0,
    then wv, then remaining wq/wk blocks, wo last — compute starts
    ~3us in while the rest of the weights stream behind it.
  - scores are computed transposed ([k, q], keys on partitions) into
    two PSUM banks per head (k-chunks 0+2 packed into one 512-col
    bank) -> 2 exps per head instead of 3. No max-subtraction needed
    (scores are O(1); masked entries get -1e4 -> exp == 0).
  - attn@V contracts over k on partitions with a fused ones-column in
    each V tile producing the softmax denominator as PSUM row 64; one
    [65, 384] copy moves attn+denominator to SBUF together.
  - Denominators DMA-scatter to an [8, 48] block per head so the DVE
    reciprocal runs on 32 full partitions per 4-head group (~0.4us
    instead of 2.5us on 12 lanes), then a K=1 PE matmul broadcasts
    each row across 64 partitions for the normalize multiply.
  - Q/K projections interleave with attention per 2-head chunk so the
    PE never waits on a cold weight block.
"""

import numpy as np

B, C, S, H, D = 8, 768, 384, 12, 64
NC = C // 128  # 6
NEG = -10000.0

_STATE = {}


# --------------------------------------------------------------------------
# Workaround: this walrus build rejects the multi-wait InstDrain that
# TileContext emits at exit ("Too many sync wait commands"). Split the
# drain's sem waits onto standalone sync-engine wait instructions.
def _patch_tile_drain():
    import concourse.tile as tile_mod
    from concourse.vector_clock import ScopedClock
    from bass_rust import SyncInfo

    if getattr(tile_mod.TileContext, "_drain_split_patch", False):
        return

    def _drain_and_barrier_split(self, tick_clock, wait_clock):
        nc = self.nc
        assert self.sems is not None
        handles = {}
        for h in self.sems.allocated().values():
            handles[h.num] = h
            handles[h.name] = h

        probe = nc.sync.nop()
        wait_clock.add_sem_waits(
            probe.ins, ScopedClock({None: tick_clock.global_clock})
        )
        waits = list(probe.ins.sync_info.on_wait)
        probe.ins.sync_info = SyncInfo(on_wait=[], on_update=[])
        for w in waits:
            h = handles.get(w.id) or handles.get(w.ant_name)
            if h is not None:
                nc.sync.wait_ge(h, w.wait_value)
            else:
                n2 = nc.sync.nop()
                n2.ins.sync_info = SyncInfo(on_wait=[w], on_update=[])

        drain_inst = nc.sync.drain()
        wait_clock.add_sem_waits(
            drain_inst.ins, ScopedClock({None: tick_clock.global_clock})
        )
        if list(drain_inst.ins.sync_info.on_wait):
            drain_inst.ins.sync_info = SyncInfo(on_wait=[], on_update=[])

        nc.all_engine_barrier()
        popped = nc._tile_sem_poison_stack.pop()
        assert popped is self._sem_poison
        nc.clear_and_free_semaphores(list(self.sems.allocated().values()))
        nc.all_engine_barrier()

        # This walrus codegen supports at most ONE sem wait per
        # instruction. Move extra waits onto same-engine nop carriers
        # inserted just before the instruction (engine queues execute in
        # order, so the semantics are identical).
        import concourse.mybir as mybir

        k = 0
        for f in nc.m.functions:
            for bb in f.blocks:
                new_insts = []
                for inst in bb.instructions:
                    si = inst.sync_info
                    waits = list(si.on_wait) if si else []
                    if len(waits) > 1:
                        for w in waits[:-1]:
                            nop = mybir.InstNoOp(name=f"I-wsplit-{k}")
                            k += 1
                            nop.engine = inst.engine
                            nop.sync_info = SyncInfo(on_wait=[w], on_update=[])
                            nc.register_instruction(nop)
                            new_insts.append(nop)
                        inst.sync_info = SyncInfo(
                            on_wait=[waits[-1]], on_update=list(si.on_update)
                        )
                    new_insts.append(inst)
                bb.instructions = new_insts

    tile_mod.TileContext._drain_and_barrier = _drain_and_barrier_split
    tile_mod.TileContext._drain_split_patch = True


# --------------------------------------------------------------------------
def _build_nc():
    import concourse.bass as bass
    import concourse.mybir as mybir
    import concourse.tile as tile

    _patch_tile_drain()

    f32 = mybir.dt.float32
    f32r = mybir.dt.float32r
    bf16 = mybir.dt.bfloat16
    Ident = mybir.ActivationFunctionType.Identity
    Exp = mybir.ActivationFunctionType.Exp

    nc = bass.Bass()
    # host-packed inputs (see _prep_maps for the exact layouts)
    x_d = nc.dram_tensor("xp", [128, NC * S], bf16, kind="ExternalInput")
    wq_d = nc.dram_tensor("wqp", [NC, 128, C], bf16, kind="ExternalInput")
    wk_d = nc.dram_tensor("wkp", [NC, 128, C], bf16, kind="ExternalInput")
    wo_d = nc.dram_tensor("wop", [NC, 128, C], bf16, kind="ExternalInput")
    wv_d = nc.dram_tensor("wvp", [128, NC * C], bf16, kind="ExternalInput")
    # consts: cols 0:6 bq, 6:12 bk, 12:18 obias, 18:146 triangular mask
    cb_d = nc.dram_tensor("cb", [128, 146], f32, kind="ExternalInput")
    y_d = nc.dram_tensor("y", [C, S], f32, kind="ExternalOutput")

    with tile.TileContext(nc) as tc:
        with (
            tc.tile_pool(name="persist", bufs=1) as persist,
            tc.tile_pool(name="epool", bufs=6) as epool,
            tc.tile_pool(name="opool", bufs=3) as opool,
            tc.tile_pool(name="psP", bufs=2, space="PSUM") as psP,
            tc.tile_pool(name="psS", bufs=3, space="PSUM") as psS,
            tc.tile_pool(name="psV", bufs=2, space="PSUM") as psV,
            tc.tile_pool(name="psR", bufs=1, space="PSUM") as psR,
        ):
            # ---- persistent tiles -------------------------------------
            xt = persist.tile([128, NC, S], bf16, tag="x", name="x")
            wvb = persist.tile([128, NC, C], bf16, tag="wv", name="wv")
            wqb = [
                persist.tile([128, NC, 128], bf16, tag=f"wq{i}", name=f"wq{i}")
                for i in range(NC)
            ]
            wkb = [
                persist.tile([128, NC, 128], bf16, tag=f"wk{i}", name=f"wk{i}")
                for i in range(NC)
            ]
            wob = [
                persist.tile([128, NC, 128], bf16, tag=f"wo{i}", name=f"wo{i}")
                for i in range(NC)
            ]
            cb = persist.tile([128, 146], f32, tag="cb", name="cb")
            q_sb = [
                persist.tile([128, S], bf16, tag=f"q{i}", name=f"q{i}")
                for i in range(NC)
            ]
            k_sb = [
                persist.tile([128, S], bf16, tag=f"k{i}", name=f"k{i}")
                for i in range(NC)
            ]
            vt = [
                persist.tile([128, H, D + 1], bf16, tag=f"vt{sq}", name=f"vt{sq}")
                for sq in range(3)
            ]
            au = [
                persist.tile([D + 1, S], f32, tag=f"au{h}", name=f"au{h}")
                for h in range(H)
            ]
            attn_sb = [
                persist.tile([128, S], bf16, tag=f"at{i}", name=f"at{i}")
                for i in range(NC)
            ]
            # head h's denominator row lives at partitions
            # (h//4)*32 + (h%4)*8 .. +8, 48 q-values per partition
            sums_sb = persist.tile([128, 48], f32, tag="sums", name="sums")
            rinv_sb = persist.tile([128, 48], f32, tag="rinv", name="rinv")
            rinv_r = persist.tile([1, H, S], f32r, tag="rinvr", name="rinvr")
            ones_sb = persist.tile([1, D], f32, tag="ones", name="ones")

            # ---- DMA issue (3 queues; per-queue order = priority) -----
            # scalar: x/3, wq0-2, wv[0:2], wq3-5
            # sync:   x/3, wk0-2, wv[2:4], wk3-5, wo0-5, y-out
            # gpsimd: x/3, consts, wv[4:6], sums scatters, rinv repacks
            third = 2 * S
            nc.scalar.dma_start(xt[:, 0:2, :], x_d[:, 0:third])
            nc.sync.dma_start(xt[:, 2:4, :], x_d[:, third : 2 * third])
            nc.gpsimd.dma_start(xt[:, 4:6, :], x_d[:, 2 * third : 3 * third])
            nc.gpsimd.dma_start(cb[:], cb_d[:, :])
            for i in range(3):
                nc.scalar.dma_start(wqb[i][:], wq_d[i])
                nc.sync.dma_start(wkb[i][:], wk_d[i])
            nc.scalar.dma_start(wvb[:, 0:2, :], wv_d[:, 0 : 2 * C])
            nc.sync.dma_start(wvb[:, 2:4, :], wv_d[:, 2 * C : 4 * C])
            nc.gpsimd.dma_start(wvb[:, 4:6, :], wv_d[:, 4 * C : 6 * C])
            for i in range(3, NC):
                nc.scalar.dma_start(wqb[i][:], wq_d[i])
                nc.sync.dma_start(wkb[i][:], wk_d[i])
            for i in range(NC):
                nc.sync.dma_start(wob[i][:], wo_d[i])

            nc.vector.memset(ones_sb[:], 1.0)
            for sq in range(3):
                nc.vector.memset(vt[sq][:, :, D : D + 1], 1.0)

            mask = cb[:, 18:146]

            # ---- building blocks --------------------------------------
            def qk_proj(oc, wtiles, bias_col, out):
                ps = psP.tile([128, S], f32, tag="proj", name="proj")
                for cc in range(NC):
                    nc.tensor.matmul(
                        ps[:],
                        wtiles[oc][:, cc, :],
                        xt[:, cc, :],
                        start=(cc == 0),
                        stop=(cc == NC - 1),
                    )
                nc.scalar.activation(
                    out[:], ps[:], Ident, bias=cb[:, bias_col : bias_col + 1]
                )

            def v_proj(half):
                # vt[sq][:, half*6:(half+1)*6, 0:64] = (X^T Wv')[s, c' half]
                # cc order follows wv chunk DMA arrival (gpsimd half
                # lands first, then scalar/sync halves)
                cc_order = [4, 5, 0, 1, 2, 3]
                for sq in range(3):
                    ps = psP.tile([128, S], f32, tag="proj", name="proj")
                    for step, cc in enumerate(cc_order):
                        nc.tensor.matmul(
                            ps[:],
                            xt[:, cc, sq * 128 : (sq + 1) * 128],
                            wvb[:, cc, half * 384 : (half + 1) * 384],
                            start=(step == 0),
                            stop=(step == NC - 1),
                        )
                    nc.vector.tensor_copy(
                        vt[sq][:, half * 6 : (half + 1) * 6, 0:D],
                        ps[:].rearrange("p (h d) -> p h d", d=D),
                    )

            def head_scores(h):
                # A = [kc0 (q 0:128) | kc2 (q 0:384)], B = [kc1 (q 0:256)]
                oc, prow = h // 2, (h % 2) * D
                Qh = q_sb[oc][prow : prow + D, :]
                Kh = k_sb[oc][prow : prow + D, :]
                ps_a = psS.tile([128, 512], f32, tag="s", name="sa")
                ps_b = psS.tile([128, 512], f32, tag="s", name="sb")
                nc.tensor.matmul(
                    ps_a[:, 0:128], Kh[:, 0:128], Qh[:, 0:128],
                    start=True, stop=True,
                )
                nc.tensor.matmul(
                    ps_a[:, 128:512], Kh[:, 256:384], Qh[:, 0:384],
                    start=True, stop=True, skip_group_check=True,
                )
                nc.tensor.matmul(
                    ps_b[:, 0:256], Kh[:, 128:256], Qh[:, 0:256],
                    start=True, stop=True,
                )
                # causal mask: only each k-chunk's diagonal block
                nc.vector.tensor_add(ps_a[:, 0:128], ps_a[:, 0:128], mask)
                nc.vector.tensor_add(ps_a[:, 384:512], ps_a[:, 384:512], mask)
                nc.vector.tensor_add(ps_b[:, 128:256], ps_b[:, 128:256], mask)
                eA = epool.tile([128, 512], bf16, tag="eA", name="eA")
                eB = epool.tile([128, 256], bf16, tag="eB", name="eB")
                nc.scalar.activation(eB[:], ps_b[:, 0:256], Exp)
                nc.scalar.activation(eA[:], ps_a[:], Exp)
                return eA, eB

            def head_av(h, eA, eB):
                # attn@V with fused denominator column; accumulate widest
                # first so every element's first write carries start
                ps_av = psV.tile([D + 1, S], f32, tag="av", name="av")
                nc.tensor.matmul(
                    ps_av[:, 0:384], vt[2][:, h, :], eA[:, 128:512],
                    start=True, stop=False, skip_group_check=True,
                )
                nc.tensor.matmul(
                    ps_av[:, 0:256], vt[1][:, h, :], eB[:, 0:256],
                    start=False, stop=False, skip_group_check=True,
                )
                nc.tensor.matmul(
                    ps_av[:, 0:128], vt[0][:, h, :], eA[:, 0:128],
                    start=False, stop=True, skip_group_check=True,
                )
                nc.vector.tensor_copy(au[h][:], ps_av[:])
                base = (h // 4) * 32 + (h % 4) * 8
                nc.gpsimd.dma_start(
                    sums_sb[base : base + 8, :], au[h][D : D + 1, :]
                )

            def normalize_group(g):
                # heads 4g..4g+3: one 32-partition reciprocal, repack to
                # [1, 4, 384] (f32r view), K=1 matmuls broadcast each row
                # across 64 partitions for the normalize multiply
                p0 = 32 * g
                nc.vector.reciprocal(
                    rinv_sb[p0 : p0 + 32, :], sums_sb[p0 : p0 + 32, :]
                )
                nc.gpsimd.dma_start(
                    rinv_r[0:1, 4 * g : 4 * g + 4, :],
                    rinv_sb[p0 : p0 + 32, :].bitcast(f32r),
                )
                for h in range(4 * g, 4 * g + 4):
                    oc, prow = h // 2, (h % 2) * D
                    ps_r = psR.tile([D, S], f32, tag="rb", name="rb")
                    nc.tensor.matmul(
                        ps_r[:], ones_sb[:].bitcast(f32r), rinv_r[0:1, h, :],
                        start=True, stop=True,
                    )
                    nc.vector.tensor_mul(
                        attn_sb[oc][prow : prow + D, :], au[h][0:D, :], ps_r[:]
                    )

            # ---- schedule ---------------------------------------------
            # Q/K projections and scores run ahead while wv streams in;
            # attn@V starts once the V projection lands.
            es = {}
            for blk in range(3):
                qk_proj(blk, wqb, blk, q_sb[blk])
                qk_proj(blk, wkb, 6 + blk, k_sb[blk])
                es[2 * blk] = head_scores(2 * blk)
                es[2 * blk + 1] = head_scores(2 * blk + 1)
            v_proj(0)
            v_proj(1)
            for h in range(4):
                head_av(h, *es.pop(h))
            normalize_group(0)
            qk_proj(3, wqb, 3, q_sb[3])
            qk_proj(3, wkb, 9, k_sb[3])
            es[6] = head_scores(6)
            es[7] = head_scores(7)
            for h in range(4, 8):
                head_av(h, *es.pop(h))
            normalize_group(1)
            for blk in range(4, NC):
                qk_proj(blk, wqb, blk, q_sb[blk])
                qk_proj(blk, wkb, 6 + blk, k_sb[blk])
                es[2 * blk] = head_scores(2 * blk)
                es[2 * blk + 1] = head_scores(2 * blk + 1)
            for h in range(8, 12):
                head_av(h, *es.pop(h))
            normalize_group(2)

            # ---- output projection (bias = host-folded Wo @ bv) -------
            for oc in range(NC):
                ps = psP.tile([128, S], f32, tag="proj", name="proj")
                for cc in range(NC):
                    nc.tensor.matmul(
                        ps[:],
                        wob[oc][:, cc, :],
                        attn_sb[cc][:],
                        start=(cc == 0),
                        stop=(cc == NC - 1),
                    )
                ot = opool.tile([128, S], f32, tag="o", name="o")
                nc.scalar.activation(
                    ot[:], ps[:], Ident, bias=cb[:, 12 + oc : 13 + oc]
                )
                nc.sync.dma_start(y_d[oc * 128 : (oc + 1) * 128, :], ot[:])

    return nc


def _get_nc():
    if "nc" not in _STATE:
        _STATE["nc"] = _build_nc()
    return _STATE["nc"]


# --------------------------------------------------------------------------
def _prep_maps(inputs):
    import ml_dtypes

    bf16 = ml_dtypes.bfloat16
    hs = np.asarray(inputs["hidden_states"], dtype=np.float32)
    Wq = np.asarray(inputs["Wq"], dtype=np.float32)
    bq = np.asarray(inputs["bq"], dtype=np.float32)
    Wk = np.asarray(inputs["Wk"], dtype=np.float32)
    bk = np.asarray(inputs["bk"], dtype=np.float32)
    Wv = np.asarray(inputs["Wv"], dtype=np.float32)
    bv = np.asarray(inputs["bv"], dtype=np.float32)
    Wo = np.asarray(inputs["Wo"], dtype=np.float32)

    # head-major channel permutation: c' = h*64 + d  <-  c = d*12 + h
    idx = (np.arange(H)[:, None] + np.arange(D)[None, :] * H).reshape(C)
    scale = float(D) ** -0.5

    def pack_blocks(wt):
        # wt: [c_in, c_out] -> [oc, p, cc*128 + co]
        w4 = wt.reshape(NC, 128, NC, 128).transpose(2, 1, 0, 3)
        return np.ascontiguousarray(w4.reshape(NC, 128, C)).astype(bf16)

    wqp = pack_blocks((scale * Wq[idx, :]).T)
    wkp = pack_blocks(Wk[idx, :].T)
    wop = pack_blocks(Wo.T)
    # wv: [p, cc*768 + co]
    wvp = np.ascontiguousarray(
        Wv[idx, :].T.reshape(NC, 128, C).transpose(1, 0, 2).reshape(128, NC * C)
    ).astype(bf16)

    cbm = np.zeros((128, 146), dtype=np.float32)
    cbm[:, 0:6] = (scale * bq[idx]).reshape(6, 128).T
    cbm[:, 6:12] = bk[idx].reshape(6, 128).T
    # V-bias folded through attention (softmax rows sum to 1):
    # attn' = attn_nobias' + bv[idx], so out += Wo @ bv[idx]
    cbm[:, 12:18] = (Wo @ bv[idx]).reshape(6, 128).T
    # mask[k, q] = NEG where k < q within a diagonal 128-block
    cbm[:, 18:146] = np.triu(np.full((128, 128), NEG, dtype=np.float32), 1)

    shared = {"wqp": wqp, "wkp": wkp, "wop": wop, "wvp": wvp, "cb": cbm}
    maps = []
    for b in range(B):
        xb = hs[b, :, 0, :].reshape(NC, 128, S).transpose(1, 0, 2)
        xp = np.ascontiguousarray(xb.reshape(128, NC * S)).astype(bf16)
        maps.append({"xp": xp, **shared})
    return maps


def _run(inputs, trace=False, **kwargs):
    from concourse.bass_utils import run_bass_kernel_spmd

    nc = _get_nc()
    in_maps = _prep_maps(inputs)
    res = run_bass_kernel_spmd(
        nc, in_maps, core_ids=list(range(B)), trace=trace, **kwargs
    )
    out = np.stack([res.results[b]["y"] for b in range(B)], axis=0)
    return out.reshape(B, C, 1, S).astype(np.float32), res


def kernel(**inputs):
    out, _ = _run(inputs, trace=False)
    return out


# revision 15
# speedup vs baseline: 1.2523x; 1.0185x over previous
"""Trainium2 Bass kernel for nn_Attention_30468497997979.

Reference computation (per batch b of 8):
    X = hidden_states[b,:,0,:]              # (C=768, S=384)
    Q/K/V = W @ X + b                       # 1x1 conv == channel matmul
    per head h (12 heads, head dim 64, channel c = d*12 + h):
        scores = (Q_h^T K_h) / 8, mask q>k, softmax over k
        attn_h = V_h @ softmax
    out = Wo @ concat_heads(attn)           # channel c = h*64 + d
Sharding: pure data-parallel, one batch per NeuronCore (8 cores).

Per-core kernel design (v2 — DMA/schedule-optimized):
  - All matmul data is bf16 (1 PE col/cycle); PSUM accumulation fp32.
  - Host pre-permutes W_{q,k,v} rows to head-major channel order
    (c' = h*64 + d), transposes to [c_in, c_out], folds 1/sqrt(d) into
    Wq/bq, folds the V bias through attention (softmax rows sum to 1)
    into an output bias Wo @ bv.
  - Weights are host-packed so every SBUF tile is one contiguous
    hardware-DMA: wq/wk/wo as six per-output-block [128, 6*128] loads
    (the kernel consumes output blocks one at a time, so each block
    gets its own completion semaphore), wv/x as [p][chunk][col] packs.
  - DMA priority: x split across three queues first, then wq0/wk0,
    then wv, then remaining wq/wk blocks, wo last — compute starts
    ~3us in while the rest of the weights stream behind it.
  - scores are computed transposed ([k, q], keys on partitions) into
    two PSUM banks per head (k-chunks 0+2 packed into one 512-col
    bank) -> 2 exps per head instead of 3. No max-subtraction needed
    (scores are O(1); masked entries get -1e4 -> exp == 0).
  - attn@V contracts over k on partitions with a fused ones-column in
    each V tile producing the softmax denominator as PSUM row 64; one
    [65, 384] copy moves attn+denominator to SBUF together.
  - Denominators DMA-scatter to an [8, 48] block per head so the DVE
    reciprocal runs on 32 full partitions per 4-head group (~0.4us
    instead of 2.5us on 12 lanes), then a K=1 PE matmul broadcasts
    each row across 64 partitions for the normalize multiply.
  - Q/K projections interleave with attention per 2-head chunk so the
    PE never waits on a cold weight block.
"""

import numpy as np

B, C, S, H, D = 8, 768, 384, 12, 64
NC = C // 128  # 6
NEG = -10000.0

_STATE = {}


# --------------------------------------------------------------------------
# Workaround: this walrus build rejects the multi-wait InstDrain that
# TileContext emits at exit ("Too many sync wait commands"). Split the
# drain's sem waits onto standalone sync-engine wait instructions.
def _patch_tile_drain():
    import concourse.tile as tile_mod
    from concourse.vector_clock import ScopedClock
    from bass_rust import SyncInfo

    if getattr(tile_mod.TileContext, "_drain_split_patch", False):
        return

    def _drain_and_barrier_split(self, tick_clock, wait_clock):
        nc = self.nc
        assert self.sems is not None
        handles = {}
        for h in self.sems.allocated().values():
            handles[h.num] = h
            handles[h.name] = h

        probe = nc.sync.nop()
        wait_clock.add_sem_waits(
            probe.ins, ScopedClock({None: tick_clock.global_clock})
        )
        waits = list(probe.ins.sync_info.on_wait)
        probe.ins.sync_info = SyncInfo(on_wait=[], on_update=[])
        for w in waits:
            h = handles.get(w.id) or handles.get(w.ant_name)
            if h is not None:
                nc.sync.wait_ge(h, w.wait_value)
            else:
                n2 = nc.sync.nop()
                n2.ins.sync_info = SyncInfo(on_wait=[w], on_update=[])

        drain_inst = nc.sync.drain()
        wait_clock.add_sem_waits(
            drain_inst.ins, ScopedClock({None: tick_clock.global_clock})
        )
        if list(drain_inst.ins.sync_info.on_wait):
            drain_inst.ins.sync_info = SyncInfo(on_wait=[], on_update=[])

        nc.all_engine_barrier()
        popped = nc._tile_sem_poison_stack.pop()
        assert popped is self._sem_poison
        nc.clear_and_free_semaphores(list(self.sems.allocated().values()))
        nc.all_engine_barrier()

        # This walrus codegen supports at most ONE sem wait per
        # instruction. Move extra waits onto same-engine nop carriers
        # inserted just before the instruction (engine queues execute in
        # order, so the semantics are identical).
        import concourse.mybir as mybir

        k = 0
        for f in nc.m.functions:
            for bb in f.blocks:
                new_insts = []
                for inst in bb.instructions:
                    si = inst.sync_info
                    waits = list(si.on_wait) if si else []
                    if len(waits) > 1:
                        for w in waits[:-1]:
                            nop = mybir.InstNoOp(name=f"I-wsplit-{k}")
                            k += 1
                            nop.engine = inst.engine
                            nop.sync_info = SyncInfo(on_wait=[w], on_update=[])
                            nc.register_instruction(nop)
                            new_insts.append(nop)
                        inst.sync_info = SyncInfo(
                            on_wait=[waits[-1]], on_update=list(si.on_update)
                        )
                    new_insts.append(inst)
                bb.instructions = new_insts

    tile_mod.TileContext._drain_and_barrier = _drain_and_barrier_split
    tile_mod.TileContext._drain_split_patch = True


# --------------------------------------------------------------------------
def _build_nc():
    import concourse.bass as bass
    import concourse.mybir as mybir
    import concourse.tile as tile

    _patch_tile_drain()

    f32 = mybir.dt.float32
    f32r = mybir.dt.float32r
    bf16 = mybir.dt.bfloat16
    Ident = mybir.ActivationFunctionType.Identity
    Exp = mybir.ActivationFunctionType.Exp

    nc = bass.Bass()
    # host-packed inputs (see _prep_maps for the exact layouts)
    x_d = nc.dram_tensor("xp", [128, NC * S], bf16, kind="ExternalInput")
    wq_d = nc.dram_tensor("wqp", [NC, 128, C], bf16, kind="ExternalInput")
    wk_d = nc.dram_tensor("wkp", [NC, 128, C], bf16, kind="ExternalInput")
    wo_d = nc.dram_tensor("wop", [NC, 128, C], bf16, kind="ExternalInput")
    wv_d = nc.dram_tensor("wvp", [128, NC * C], bf16, kind="ExternalInput")
    # consts: cols 0:6 bq, 6:12 bk, 12:18 obias, 18:146 triangular mask
    cb_d = nc.dram_tensor("cb", [128, 146], f32, kind="ExternalInput")
    y_d = nc.dram_tensor("y", [C, S], f32, kind="ExternalOutput")

    with tile.TileContext(nc) as tc:
        with (
            tc.tile_pool(name="persist", bufs=1) as persist,
            tc.tile_pool(name="epool", bufs=6) as epool,
            tc.tile_pool(name="opool", bufs=3) as opool,
            tc.tile_pool(name="psP", bufs=2, space="PSUM") as psP,
            tc.tile_pool(name="psS", bufs=3, space="PSUM") as psS,
            tc.tile_pool(name="psV", bufs=2, space="PSUM") as psV,
            tc.tile_pool(name="psR", bufs=1, space="PSUM") as psR,
        ):
            # ---- persistent tiles -------------------------------------
            xt = persist.tile([128, NC, S], bf16, tag="x", name="x")
            wvb = persist.tile([128, NC, C], bf16, tag="wv", name="wv")
            wqb = [
                persist.tile([128, NC, 128], bf16, tag=f"wq{i}", name=f"wq{i}")
                for i in range(NC)
            ]
            wkb = [
                persist.tile([128, NC, 128], bf16, tag=f"wk{i}", name=f"wk{i}")
                for i in range(NC)
            ]
            wob = [
                persist.tile([128, NC, 128], bf16, tag=f"wo{i}", name=f"wo{i}")
                for i in range(NC)
            ]
            cb = persist.tile([128, 146], f32, tag="cb", name="cb")
            q_sb = [
                persist.tile([128, S], bf16, tag=f"q{i}", name=f"q{i}")
                for i in range(NC)
            ]
            k_sb = [
                persist.tile([128, S], bf16, tag=f"k{i}", name=f"k{i}")
                for i in range(NC)
            ]
            vt = [
                persist.tile([128, H, D + 1], bf16, tag=f"vt{sq}", name=f"vt{sq}")
                for sq in range(3)
            ]
            au = [
                persist.tile([D + 1, S], f32, tag=f"au{h}", name=f"au{h}")
                for h in range(H)
            ]
            attn_sb = [
                persist.tile([128, S], bf16, tag=f"at{i}", name=f"at{i}")
                for i in range(NC)
            ]
            # head h's denominator row lives at partitions
            # (h//4)*32 + (h%4)*8 .. +8, 48 q-values per partition
            sums_sb = persist.tile([128, 48], f32, tag="sums", name="sums")
            rinv_sb = persist.tile([128, 48], f32, tag="rinv", name="rinv")
            rinv_r = persist.tile([1, H, S], f32r, tag="rinvr", name="rinvr")
            ones_sb = persist.tile([1, D], f32, tag="ones", name="ones")

            # ---- DMA issue --------------------------------------------
            # Only the Sync and Activation queues are hardware-dynamic
            # (~134 GB/s each); the GpSimd queue is software-dynamic at
            # ~13 GB/s aggregate, so it only carries the tiny SBUF->SBUF
            # denominator moves. Per-queue order = priority (need order).
            # The Activation engine also runs the proj-copy/exp pipeline,
            # so only its startup-critical DMAs are issued up front; the
            # rest are issued from mid-program points (the queue keeps
            # streaming earlier transfers meanwhile).
            nc.scalar.dma_start(cb[:], cb_d[:, :])
            nc.scalar.dma_start(xt[:, 0:3, :], x_d[:, 0 : 3 * S])
            nc.scalar.dma_start(wqb[0][:], wq_d[0])
            nc.scalar.dma_start(wqb[1][:], wq_d[1])
            nc.sync.dma_start(xt[:, 3:6, :], x_d[:, 3 * S : 6 * S])
            nc.sync.dma_start(wkb[0][:], wk_d[0])
            nc.sync.dma_start(wkb[1][:], wk_d[1])
            nc.sync.dma_start(wvb[:, 3:6, :], wv_d[:, 3 * C : 6 * C])
            for i in range(2, NC):
                nc.sync.dma_start(wkb[i][:], wk_d[i])
            for i in range(NC):
                nc.sync.dma_start(wob[i][:], wo_d[i])

            nc.vector.memset(ones_sb[:], 1.0)
            for sq in range(3):
                nc.vector.memset(vt[sq][:, :, D : D + 1], 1.0)

            mask = cb[:, 18:146]

            # ---- building blocks --------------------------------------
            def qk_proj(oc, wtiles, bias_col, out):
                ps = psP.tile([128, S], f32, tag="proj", name="proj")
                for cc in range(NC):
                    nc.tensor.matmul(
                        ps[:],
                        wtiles[oc][:, cc, :],
                        xt[:, cc, :],
                        start=(cc == 0),
                        stop=(cc == NC - 1),
                    )
                nc.scalar.activation(
                    out[:], ps[:], Ident, bias=cb[:, bias_col : bias_col + 1]
                )

            def v_proj(half):
                # vt[sq][:, half*6:(half+1)*6, 0:64] = (X^T Wv')[s, c' half]
                # cc order follows wv chunk DMA arrival (sync half 3:6
                # lands first, then the deferred scalar half 0:3)
                cc_order = [3, 4, 5, 0, 1, 2]
                for sq in range(3):
                    ps = psP.tile([128, S], f32, tag="proj", name="proj")
                    for step, cc in enumerate(cc_order):
                        nc.tensor.matmul(
                            ps[:],
                            xt[:, cc, sq * 128 : (sq + 1) * 128],
                            wvb[:, cc, half * 384 : (half + 1) * 384],
                            start=(step == 0),
                            stop=(step == NC - 1),
                        )
                    nc.vector.tensor_copy(
                        vt[sq][:, half * 6 : (half + 1) * 6, 0:D],
                        ps[:].rearrange("p (h d) -> p h d", d=D),
                    )

            def head_scores(h):
                # A = [kc0 (q 0:128) | kc2 (q 0:384)], B = [kc1 (q 0:256)]
                oc, prow = h // 2, (h % 2) * D
                Qh = q_sb[oc][prow : prow + D, :]
                Kh = k_sb[oc][prow : prow + D, :]
                ps_a = psS.tile([128, 512], f32, tag="s", name="sa")
                ps_b = psS.tile([128, 512], f32, tag="s", name="sb")
                nc.tensor.matmul(
                    ps_a[:, 0:128], Kh[:, 0:128], Qh[:, 0:128],
                    start=True, stop=True,
                )
                nc.tensor.matmul(
                    ps_a[:, 128:512], Kh[:, 256:384], Qh[:, 0:384],
                    start=True, stop=True, skip_group_check=True,
                )
                nc.tensor.matmul(
                    ps_b[:, 0:256], Kh[:, 128:256], Qh[:, 0:256],
                    start=True, stop=True,
                )
                # causal mask: only each k-chunk's diagonal block
                nc.vector.tensor_add(ps_a[:, 0:128], ps_a[:, 0:128], mask)
                nc.vector.tensor_add(ps_a[:, 384:512], ps_a[:, 384:512], mask)
                nc.vector.tensor_add(ps_b[:, 128:256], ps_b[:, 128:256], mask)
                eA = epool.tile([128, 512], bf16, tag="eA", name="eA")
                eB = epool.tile([128, 256], bf16, tag="eB", name="eB")
                nc.scalar.activation(eB[:], ps_b[:, 0:256], Exp)
                nc.scalar.activation(eA[:], ps_a[:], Exp)
                return eA, eB

            def head_av(h, eA, eB):
                # attn@V with fused denominator column; accumulate widest
                # first so every element's first write carries start
                ps_av = psV.tile([D + 1, S], f32, tag="av", name="av")
                nc.tensor.matmul(
                    ps_av[:, 0:384], vt[2][:, h, :], eA[:, 128:512],
                    start=True, stop=False, skip_group_check=True,
                )
                nc.tensor.matmul(
                    ps_av[:, 0:256], vt[1][:, h, :], eB[:, 0:256],
                    start=False, stop=False, skip_group_check=True,
                )
                nc.tensor.matmul(
                    ps_av[:, 0:128], vt[0][:, h, :], eA[:, 0:128],
                    start=False, stop=True, skip_group_check=True,
                )
                nc.vector.tensor_copy(au[h][:], ps_av[:])
                nc.gpsimd.dma_start(
                    sums_sb[_base(h) : _base(h) + 8, :], au[h][D : D + 1, :]
                )

            # normalize groups: heads (0-3), (4-7), (8-9), (10-11) at
            # partition bases 0/32/64/96 (compute ops need 32-aligned
            # partition starts)
            GRP = [range(0, 4), range(4, 8), range(8, 10), range(10, 12)]

            def _base(h):
                g = h // 4 if h < 8 else 2 + (h - 8) // 2
                i = h - GRP[g][0]
                return 32 * g + 8 * i

            def norm_pre(g):
                # per-group reciprocal on full partitions, then repack
                # each head's row into rinv_r's free dim (f32r view)
                hs = GRP[g]
                p0, np_ = 32 * g, 8 * len(hs)
                nc.vector.reciprocal(
                    rinv_sb[p0 : p0 + np_, :], sums_sb[p0 : p0 + np_, :]
                )
                nc.gpsimd.dma_start(
                    rinv_r[0:1, hs[0] : hs[0] + len(hs), :],
                    rinv_sb[p0 : p0 + np_, :].bitcast(f32r),
                )

            def norm_bcast(h):
                # K=1 matmul broadcasts 1/sum across 64 partitions, then
                # the DVE multiply writes the normalized bf16 attn chunk
                oc, prow = h // 2, (h % 2) * D
                ps_r = psR.tile([D, S], f32, tag="rb", name="rb")
                nc.tensor.matmul(
                    ps_r[:], ones_sb[:].bitcast(f32r), rinv_r[0:1, h, :],
                    start=True, stop=True,
                )
                nc.vector.tensor_mul(
                    attn_sb[oc][prow : prow + D, :], au[h][0:D, :], ps_r[:]
                )

            # ---- schedule ---------------------------------------------
            # Q/K projections and scores run ahead while wv streams in;
            # attn@V starts once the V projection lands. Remaining weight
            # DMAs issue from mid-program so the Activation engine's FIFO
            # stays responsive for the proj-copy/exp pipeline. Normalize
            # broadcasts interleave with the next chunk's matmuls to hide
            # the denominator chain (copy->DMA->recip->DMA) latency.
            es = {}
            qk_proj(0, wqb, 0, q_sb[0])
            nc.scalar.dma_start(wvb[:, 0:3, :], wv_d[:, 0 : 3 * C])
            qk_proj(0, wkb, 6, k_sb[0])
            es[0] = head_scores(0)
            nc.scalar.dma_start(wqb[2][:], wq_d[2])
            es[1] = head_scores(1)
            nc.scalar.dma_start(wqb[3][:], wq_d[3])
            qk_proj(1, wqb, 1, q_sb[1])
            qk_proj(1, wkb, 7, k_sb[1])
            nc.scalar.dma_start(wqb[4][:], wq_d[4])
            es[2] = head_scores(2)
            nc.scalar.dma_start(wqb[5][:], wq_d[5])
            es[3] = head_scores(3)
            qk_proj(2, wqb, 2, q_sb[2])
            qk_proj(2, wkb, 8, k_sb[2])
            es[4] = head_scores(4)
            es[5] = head_scores(5)
            v_proj(0)
            v_proj(1)
            for h in range(4):
                head_av(h, *es.pop(h))
            norm_pre(0)
            head_av(4, *es.pop(4))
            norm_bcast(0)
            head_av(5, *es.pop(5))
            norm_bcast(1)
            qk_proj(3, wqb, 3, q_sb[3])
            norm_bcast(2)
            qk_proj(3, wkb, 9, k_sb[3])
            norm_bcast(3)
            es[6] = head_scores(6)
            es[7] = head_scores(7)
            head_av(6, *es.pop(6))
            head_av(7, *es.pop(7))
            norm_pre(1)
            qk_proj(4, wqb, 4, q_sb[4])
            norm_bcast(4)
            qk_proj(4, wkb, 10, k_sb[4])
            norm_bcast(5)
            es[8] = head_scores(8)
            norm_bcast(6)
            es[9] = head_scores(9)
            norm_bcast(7)
            head_av(8, *es.pop(8))
            head_av(9, *es.pop(9))
            norm_pre(2)
            qk_proj(5, wqb, 5, q_sb[5])
            norm_bcast(8)
            qk_proj(5, wkb, 11, k_sb[5])
            norm_bcast(9)
            es[10] = head_scores(10)
            es[11] = head_scores(11)
            head_av(10, *es.pop(10))
            head_av(11, *es.pop(11))
            norm_pre(3)
            norm_bcast(10)
            norm_bcast(11)

            # ---- output projection (bias = host-folded Wo @ bv) -------
            for oc in range(NC):
                ps = psP.tile([128, S], f32, tag="proj", name="proj")
                for cc in range(NC):
                    nc.tensor.matmul(
                        ps[:],
                        wob[oc][:, cc, :],
                        attn_sb[cc][:],
                        start=(cc == 0),
                        stop=(cc == NC - 1),
                    )
                ot = opool.tile([128, S], f32, tag="o", name="o")
                nc.scalar.activation(
                    ot[:], ps[:], Ident, bias=cb[:, 12 + oc : 13 + oc]
                )
                nc.sync.dma_start(y_d[oc * 128 : (oc + 1) * 128, :], ot[:])

    return nc


def _get_nc():
    if "nc" not in _STATE:
        _STATE["nc"] = _build_nc()
    return _STATE["nc"]


# --------------------------------------------------------------------------
def _prep_maps(inputs):
    import ml_dtypes

    bf16 = ml_dtypes.bfloat16
    hs = np.asarray(inputs["hidden_states"], dtype=np.float32)
    Wq = np.asarray(inputs["Wq"], dtype=np.float32)
    bq = np.asarray(inputs["bq"], dtype=np.float32)
    Wk = np.asarray(inputs["Wk"], dtype=np.float32)
    bk = np.asarray(inputs["bk"], dtype=np.float32)
    Wv = np.asarray(inputs["Wv"], dtype=np.float32)
    bv = np.asarray(inputs["bv"], dtype=np.float32)
    Wo = np.asarray(inputs["Wo"], dtype=np.float32)

    # head-major channel permutation: c' = h*64 + d  <-  c = d*12 + h
    idx = (np.arange(H)[:, None] + np.arange(D)[None, :] * H).reshape(C)
    scale = float(D) ** -0.5

    def pack_blocks(wt):
        # wt: [c_in, c_out] -> [oc, p, cc*128 + co]
        w4 = wt.reshape(NC, 128, NC, 128).transpose(2, 1, 0, 3)
        return np.ascontiguousarray(w4.reshape(NC, 128, C)).astype(bf16)

    wqp = pack_blocks((scale * Wq[idx, :]).T)
    wkp = pack_blocks(Wk[idx, :].T)
    wop = pack_blocks(Wo.T)
    # wv: [p, cc*768 + co]
    wvp = np.ascontiguousarray(
        Wv[idx, :].T.reshape(NC, 128, C).transpose(1, 0, 2).reshape(128, NC * C)
    ).astype(bf16)

    cbm = np.zeros((128, 146), dtype=np.float32)
    cbm[:, 0:6] = (scale * bq[idx]).reshape(6, 128).T
    cbm[:, 6:12] = bk[idx].reshape(6, 128).T
    # V-bias folded through attention (softmax rows sum to 1):
    # attn' = attn_nobias' + bv[idx], so out += Wo @ bv[idx]
    cbm[:, 12:18] = (Wo @ bv[idx]).reshape(6, 128).T
    # mask[k, q] = NEG where k < q within a diagonal 128-block
    cbm[:, 18:146] = np.triu(np.full((128, 128), NEG, dtype=np.float32), 1)

    shared = {"wqp": wqp, "wkp": wkp, "wop": wop, "wvp": wvp, "cb": cbm}
    maps = []
    for b in range(B):
        xb = hs[b, :, 0, :].reshape(NC, 128, S).transpose(1, 0, 2)
        xp = np.ascontiguousarray(xb.reshape(128, NC * S)).astype(bf16)
        maps.append({"xp": xp, **shared})
    return maps


def _run(inputs, trace=False, **kwargs):
    from concourse.bass_utils import run_bass_kernel_spmd

    nc = _get_nc()
    in_maps = _prep_maps(inputs)
    res = run_bass_kernel_spmd(
        nc, in_maps, core_ids=list(range(B)), trace=trace, **kwargs
    )
    out = np.stack([res.results[b]["y"] for b in range(B)], axis=0)
    return out.reshape(B, C, 1, S).astype(np.float32), res


def kernel(**inputs):
    out, _ = _run(inputs, trace=False)
    return out


# revision 21
# speedup vs baseline: 1.3304x; 1.0624x over previous
"""Trainium2 Bass kernel for nn_Attention_30468497997979.

Reference computation (per batch b of 8):
    X = hidden_states[b,:,0,:]              # (C=768, S=384)
    Q/K/V = W @ X + b                       # 1x1 conv == channel matmul
    per head h (12 heads, head dim 64, channel c = d*12 + h):
        scores = (Q_h^T K_h) / 8, mask q>k, softmax over k
        attn_h = V_h @ softmax
    out = Wo @ concat_heads(attn)           # channel c = h*64 + d
Sharding: pure data-parallel, one batch per NeuronCore (8 cores).

Per-core kernel design (v2 — DMA/schedule-optimized):
  - All matmul data is bf16 (1 PE col/cycle); PSUM accumulation fp32.
  - Host pre-permutes W_{q,k,v} rows to head-major channel order
    (c' = h*64 + d), transposes to [c_in, c_out], folds 1/sqrt(d) into
    Wq/bq, folds the V bias through attention (softmax rows sum to 1)
    into an output bias Wo @ bv.
  - Weights are host-packed so every SBUF tile is one contiguous
    hardware-DMA: wq/wk/wo as six per-output-block [128, 6*128] loads
    (the kernel consumes output blocks one at a time, so each block
    gets its own completion semaphore), wv/x as [p][chunk][col] packs.
  - DMA priority: x split across three queues first, then wq0/wk0,
    then wv, then remaining wq/wk blocks, wo last — compute starts
    ~3us in while the rest of the weights stream behind it.
  - scores are computed transposed ([k, q], keys on partitions) into
    two PSUM banks per head (k-chunks 0+2 packed into one 512-col
    bank) -> 2 exps per head instead of 3. No max-subtraction needed
    (scores are O(1); masked entries get -1e4 -> exp == 0).
  - attn@V contracts over k on partitions with a fused ones-column in
    each V tile producing the softmax denominator as PSUM row 64; one
    [65, 384] copy moves attn+denominator to SBUF together.
  - Denominators DMA-scatter to an [8, 48] block per head so the DVE
    reciprocal runs on 32 full partitions per 4-head group (~0.4us
    instead of 2.5us on 12 lanes), then a K=1 PE matmul broadcasts
    each row across 64 partitions for the normalize multiply.
  - Q/K projections interleave with attention per 2-head chunk so the
    PE never waits on a cold weight block.
"""

import numpy as np

B, C, S, H, D = 8, 768, 384, 12, 64
NC = C // 128  # 6
NEG = -10000.0

_STATE = {}


# --------------------------------------------------------------------------
# Workaround: this walrus build rejects the multi-wait InstDrain that
# TileContext emits at exit ("Too many sync wait commands"). Split the
# drain's sem waits onto standalone sync-engine wait instructions.
def _patch_tile_drain():
    import concourse.tile as tile_mod
    from concourse.vector_clock import ScopedClock
    from bass_rust import SyncInfo

    if getattr(tile_mod.TileContext, "_drain_split_patch", False):
        return

    def _drain_and_barrier_split(self, tick_clock, wait_clock):
        nc = self.nc
        assert self.sems is not None
        handles = {}
        for h in self.sems.allocated().values():
            handles[h.num] = h
            handles[h.name] = h

        probe = nc.sync.nop()
        wait_clock.add_sem_waits(
            probe.ins, ScopedClock({None: tick_clock.global_clock})
        )
        waits = list(probe.ins.sync_info.on_wait)
        probe.ins.sync_info = SyncInfo(on_wait=[], on_update=[])
        for w in waits:
            h = handles.get(w.id) or handles.get(w.ant_name)
            if h is not None:
                nc.sync.wait_ge(h, w.wait_value)
            else:
                n2 = nc.sync.nop()
                n2.ins.sync_info = SyncInfo(on_wait=[w], on_update=[])

        drain_inst = nc.sync.drain()
        wait_clock.add_sem_waits(
            drain_inst.ins, ScopedClock({None: tick_clock.global_clock})
        )
        if list(drain_inst.ins.sync_info.on_wait):
            drain_inst.ins.sync_info = SyncInfo(on_wait=[], on_update=[])

        nc.all_engine_barrier()
        popped = nc._tile_sem_poison_stack.pop()
        assert popped is self._sem_poison
        nc.clear_and_free_semaphores(list(self.sems.allocated().values()))
        nc.all_engine_barrier()

        # This walrus codegen supports at most ONE sem wait per
        # instruction. Move extra waits onto same-engine nop carriers
        # inserted just before the instruction (engine queues execute in
        # order, so the semantics are identical).
        import concourse.mybir as mybir

        k = 0
        for f in nc.m.functions:
            for bb in f.blocks:
                new_insts = []
                for inst in bb.instructions:
                    si = inst.sync_info
                    waits = list(si.on_wait) if si else []
                    if len(waits) > 1:
                        for w in waits[:-1]:
                            nop = mybir.InstNoOp(name=f"I-wsplit-{k}")
                            k += 1
                            nop.engine = inst.engine
                            nop.sync_info = SyncInfo(on_wait=[w], on_update=[])
                            nc.register_instruction(nop)
                            new_insts.append(nop)
                        inst.sync_info = SyncInfo(
                            on_wait=[waits[-1]], on_update=list(si.on_update)
                        )
                    new_insts.append(inst)
                bb.instructions = new_insts

    tile_mod.TileContext._drain_and_barrier = _drain_and_barrier_split
    tile_mod.TileContext._drain_split_patch = True


# --------------------------------------------------------------------------
def _build_nc():
    import concourse.bass as bass
    import concourse.mybir as mybir
    import concourse.tile as tile

    _patch_tile_drain()

    f32 = mybir.dt.float32
    f32r = mybir.dt.float32r
    bf16 = mybir.dt.bfloat16
    Ident = mybir.ActivationFunctionType.Identity
    Exp = mybir.ActivationFunctionType.Exp

    nc = bass.Bass()
    # host-packed inputs (see _prep_maps for the exact layouts)
    x_d = nc.dram_tensor("xp", [128, NC * S], bf16, kind="ExternalInput")
    wq_d = nc.dram_tensor("wqp", [NC, 128, C], bf16, kind="ExternalInput")
    wk_d = nc.dram_tensor("wkp", [NC, 128, C], bf16, kind="ExternalInput")
    wo_d = nc.dram_tensor("wop", [NC, 128, C], bf16, kind="ExternalInput")
    wv_d = nc.dram_tensor("wvp", [128, NC * C], bf16, kind="ExternalInput")
    # consts: cols 0:6 bq, 6:12 bk, 12:18 obias, 18:146 triangular mask
    cb_d = nc.dram_tensor("cb", [128, 146], f32, kind="ExternalInput")
    y_d = nc.dram_tensor("y", [C, S], f32, kind="ExternalOutput")

    with tile.TileContext(nc) as tc:
        with (
            tc.tile_pool(name="persist", bufs=1) as persist,
            tc.tile_pool(name="epool", bufs=6) as epool,
            tc.tile_pool(name="opool", bufs=3) as opool,
            tc.tile_pool(name="psP", bufs=2, space="PSUM") as psP,
            tc.tile_pool(name="psS", bufs=3, space="PSUM") as psS,
            tc.tile_pool(name="psV", bufs=2, space="PSUM") as psV,
            tc.tile_pool(name="psR", bufs=1, space="PSUM") as psR,
        ):
            # ---- persistent tiles -------------------------------------
            xt = persist.tile([128, NC, S], bf16, tag="x", name="x")
            wvb = persist.tile([128, NC, C], bf16, tag="wv", name="wv")
            wqb = [
                persist.tile([128, NC, 128], bf16, tag=f"wq{i}", name=f"wq{i}")
                for i in range(NC)
            ]
            wkb = [
                persist.tile([128, NC, 128], bf16, tag=f"wk{i}", name=f"wk{i}")
                for i in range(NC)
            ]
            wob = [
                persist.tile([128, NC, 128], bf16, tag=f"wo{i}", name=f"wo{i}")
                for i in range(NC)
            ]
            cb = persist.tile([128, 146], f32, tag="cb", name="cb")
            q_sb = [
                persist.tile([128, S], bf16, tag=f"q{i}", name=f"q{i}")
                for i in range(NC)
            ]
            k_sb = [
                persist.tile([128, S], bf16, tag=f"k{i}", name=f"k{i}")
                for i in range(NC)
            ]
            vt = [
                persist.tile([128, H, D + 1], bf16, tag=f"vt{sq}", name=f"vt{sq}")
                for sq in range(3)
            ]
            au = [
                persist.tile([D + 1, S], f32, tag=f"au{h}", name=f"au{h}")
                for h in range(H)
            ]
            attn_sb = [
                persist.tile([128, S], bf16, tag=f"at{i}", name=f"at{i}")
                for i in range(NC)
            ]
            # head h's denominator row lives at partitions
            # (h//4)*32 + (h%4)*8 .. +8, 48 q-values per partition
            sums_sb = persist.tile([128, 48], f32, tag="sums", name="sums")
            rinv_sb = persist.tile([128, 48], f32, tag="rinv", name="rinv")
            rinv_r = persist.tile([1, H, S], f32r, tag="rinvr", name="rinvr")
            ones_sb = persist.tile([1, D], f32, tag="ones", name="ones")

            # ---- DMA issue --------------------------------------------
            # Only the Sync and Activation queues are hardware-dynamic
            # (~134 GB/s each); the GpSimd queue is software-dynamic at
            # ~13 GB/s aggregate, so it only carries the tiny SBUF->SBUF
            # denominator moves. Per-queue order = priority (need order).
            # The Activation engine also runs the proj-copy/exp pipeline,
            # so only its startup-critical DMAs are issued up front; the
            # rest are issued from mid-program points (the queue keeps
            # streaming earlier transfers meanwhile).
            nc.scalar.dma_start(xt[:, 0:3, :], x_d[:, 0 : 3 * S])
            nc.scalar.dma_start(wqb[0][:], wq_d[0])
            nc.scalar.dma_start(cb[:], cb_d[:, :])
            nc.scalar.dma_start(wqb[1][:], wq_d[1])
            nc.sync.dma_start(xt[:, 3:6, :], x_d[:, 3 * S : 6 * S])
            nc.sync.dma_start(wkb[0][:], wk_d[0])
            nc.sync.dma_start(wkb[1][:], wk_d[1])
            nc.sync.dma_start(wvb[:, 3:6, :], wv_d[:, 3 * C : 6 * C])
            for i in range(2, NC):
                nc.sync.dma_start(wkb[i][:], wk_d[i])
            for i in range(NC):
                nc.sync.dma_start(wob[i][:], wo_d[i])

            nc.vector.memset(ones_sb[:], 1.0)
            for sq in range(3):
                nc.vector.memset(vt[sq][:, :, D : D + 1], 1.0)

            mask = cb[:, 18:146]

            # ---- building blocks --------------------------------------
            def qk_proj(oc, wtiles, bias_col, out):
                # copy+bias on DVE (tensor_scalar add) — the Activation
                # engine's FIFO is reserved for the exp chain
                ps = psP.tile([128, S], f32, tag="proj", name="proj")
                for cc in range(NC):
                    nc.tensor.matmul(
                        ps[:],
                        wtiles[oc][:, cc, :],
                        xt[:, cc, :],
                        start=(cc == 0),
                        stop=(cc == NC - 1),
                    )
                nc.vector.tensor_scalar_add(
                    out[:], ps[:], cb[:, bias_col : bias_col + 1]
                )

            def v_proj(half):
                # vt[sq][:, half*6:(half+1)*6, 0:64] = (X^T Wv')[s, c' half]
                # cc order follows wv chunk DMA arrival (sync half 3:6
                # lands first, then the deferred scalar half 0:3)
                cc_order = [3, 4, 5, 0, 1, 2]
                for sq in range(3):
                    ps = psP.tile([128, S], f32, tag="proj", name="proj")
                    for step, cc in enumerate(cc_order):
                        nc.tensor.matmul(
                            ps[:],
                            xt[:, cc, sq * 128 : (sq + 1) * 128],
                            wvb[:, cc, half * 384 : (half + 1) * 384],
                            start=(step == 0),
                            stop=(step == NC - 1),
                        )
                    nc.vector.tensor_copy(
                        vt[sq][:, half * 6 : (half + 1) * 6, 0:D],
                        ps[:].rearrange("p (h d) -> p h d", d=D),
                    )

            def head_scores(h):
                # A = [kc0 (q 0:128) | kc2 (q 0:384)], B = [kc1 (q 0:256)]
                # B runs first so its exp clears the PSUM ring early
                oc, prow = h // 2, (h % 2) * D
                Qh = q_sb[oc][prow : prow + D, :]
                Kh = k_sb[oc][prow : prow + D, :]
                ps_b = psS.tile([128, 512], f32, tag="s", name="sb")
                ps_a = psS.tile([128, 512], f32, tag="s", name="sa")
                nc.tensor.matmul(
                    ps_b[:, 0:256], Kh[:, 128:256], Qh[:, 0:256],
                    start=True, stop=True,
                )
                # causal mask: only each k-chunk's diagonal block
                nc.vector.tensor_add(ps_b[:, 128:256], ps_b[:, 128:256], mask)
                eB = epool.tile([128, 256], bf16, tag="eB", name="eB")
                nc.scalar.activation(eB[:], ps_b[:, 0:256], Exp)
                nc.tensor.matmul(
                    ps_a[:, 0:128], Kh[:, 0:128], Qh[:, 0:128],
                    start=True, stop=True,
                )
                nc.tensor.matmul(
                    ps_a[:, 128:512], Kh[:, 256:384], Qh[:, 0:384],
                    start=True, stop=True, skip_group_check=True,
                )
                nc.vector.tensor_add(ps_a[:, 0:128], ps_a[:, 0:128], mask)
                nc.vector.tensor_add(ps_a[:, 384:512], ps_a[:, 384:512], mask)
                eA = epool.tile([128, 512], bf16, tag="eA", name="eA")
                nc.scalar.activation(eA[:], ps_a[:], Exp)
                return eA, eB

            def head_av(h, eA, eB):
                # attn@V with fused denominator column; accumulate widest
                # first so every element's first write carries start
                ps_av = psV.tile([D + 1, S], f32, tag="av", name="av")
                nc.tensor.matmul(
                    ps_av[:, 0:384], vt[2][:, h, :], eA[:, 128:512],
                    start=True, stop=False, skip_group_check=True,
                )
                nc.tensor.matmul(
                    ps_av[:, 0:256], vt[1][:, h, :], eB[:, 0:256],
                    start=False, stop=False, skip_group_check=True,
                )
                nc.tensor.matmul(
                    ps_av[:, 0:128], vt[0][:, h, :], eA[:, 0:128],
                    start=False, stop=True, skip_group_check=True,
                )
                nc.scalar.copy(au[h][:], ps_av[:])
                nc.sync.dma_start(
                    sums_sb[_base(h) : _base(h) + 8, :], au[h][D : D + 1, :]
                )

            # normalize groups: heads (0-3), (4-7), (8-9), (10-11) at
            # partition bases 0/32/64/96 (compute ops need 32-aligned
            # partition starts)
            GRP = [range(0, 4), range(4, 8), range(8, 10), range(10, 12)]

            def _base(h):
                g = h // 4 if h < 8 else 2 + (h - 8) // 2
                i = h - GRP[g][0]
                return 32 * g + 8 * i

            def norm_pre(g):
                # per-group reciprocal on full partitions, then repack
                # each head's row into rinv_r's free dim (f32r view)
                hs = GRP[g]
                p0, np_ = 32 * g, 8 * len(hs)
                nc.vector.reciprocal(
                    rinv_sb[p0 : p0 + np_, :], sums_sb[p0 : p0 + np_, :]
                )
                nc.sync.dma_start(
                    rinv_r[0:1, hs[0] : hs[0] + len(hs), :],
                    rinv_sb[p0 : p0 + np_, :].bitcast(f32r),
                )

            def norm_bcast(h):
                # K=1 matmul broadcasts 1/sum across 64 partitions, then
                # the DVE multiply writes the normalized bf16 attn chunk
                oc, prow = h // 2, (h % 2) * D
                ps_r = psR.tile([D, S], f32, tag="rb", name="rb")
                nc.tensor.matmul(
                    ps_r[:], ones_sb[:].bitcast(f32r), rinv_r[0:1, h, :],
                    start=True, stop=True,
                )
                nc.vector.tensor_mul(
                    attn_sb[oc][prow : prow + D, :], au[h][0:D, :], ps_r[:]
                )

            # ---- schedule ---------------------------------------------
            # Q/K projections and scores run ahead while wv streams in;
            # attn@V starts once the V projection lands. Remaining weight
            # DMAs issue from mid-program so the Activation engine's FIFO
            # stays responsive for the proj-copy/exp pipeline. Normalize
            # broadcasts interleave with the next chunk's matmuls to hide
            # the denominator chain (copy->DMA->recip->DMA) latency.
            es = {}
            qk_proj(0, wqb, 0, q_sb[0])
            qk_proj(0, wkb, 6, k_sb[0])
            es[0] = head_scores(0)
            nc.scalar.dma_start(wqb[2][:], wq_d[2])
            es[1] = head_scores(1)
            nc.scalar.dma_start(wqb[3][:], wq_d[3])
            qk_proj(1, wqb, 1, q_sb[1])
            nc.scalar.dma_start(wvb[:, 0:3, :], wv_d[:, 0 : 3 * C])
            qk_proj(1, wkb, 7, k_sb[1])
            nc.scalar.dma_start(wqb[4][:], wq_d[4])
            es[2] = head_scores(2)
            nc.scalar.dma_start(wqb[5][:], wq_d[5])
            es[3] = head_scores(3)
            qk_proj(2, wqb, 2, q_sb[2])
            qk_proj(2, wkb, 8, k_sb[2])
            es[4] = head_scores(4)
            es[5] = head_scores(5)
            v_proj(0)
            v_proj(1)
            for h in range(4):
                head_av(h, *es.pop(h))
            norm_pre(0)
            head_av(4, *es.pop(4))
            head_av(5, *es.pop(5))
            qk_proj(3, wqb, 3, q_sb[3])
            norm_bcast(0)
            qk_proj(3, wkb, 9, k_sb[3])
            norm_bcast(1)
            es[6] = head_scores(6)
            norm_bcast(2)
            es[7] = head_scores(7)
            norm_bcast(3)
            head_av(6, *es.pop(6))
            head_av(7, *es.pop(7))
            norm_pre(1)
            qk_proj(4, wqb, 4, q_sb[4])
            qk_proj(4, wkb, 10, k_sb[4])
            es[8] = head_scores(8)
            norm_bcast(4)
            es[9] = head_scores(9)
            norm_bcast(5)
            head_av(8, *es.pop(8))
            norm_bcast(6)
            head_av(9, *es.pop(9))
            norm_bcast(7)
            norm_pre(2)
            qk_proj(5, wqb, 5, q_sb[5])
            qk_proj(5, wkb, 11, k_sb[5])
            es[10] = head_scores(10)
            norm_bcast(8)
            es[11] = head_scores(11)
            norm_bcast(9)
            head_av(10, *es.pop(10))
            head_av(11, *es.pop(11))
            norm_pre(3)

            # ---- output projection (bias = host-folded Wo @ bv) -------
            # the first two chunks accumulate cc 0..4 before the last
            # normalize group's broadcasts, hiding that chain's latency;
            # their cc=5 step follows the group-3 multiplies
            def out_proj_mm(ps, oc, ccs, start, stop):
                for i, cc in enumerate(ccs):
                    nc.tensor.matmul(
                        ps[:],
                        wob[oc][:, cc, :],
                        attn_sb[cc][:],
                        start=(start and i == 0),
                        stop=(stop and i == len(ccs) - 1),
                    )

            def out_proj_fin(ps, oc):
                ot = opool.tile([128, S], f32, tag="o", name="o")
                nc.scalar.activation(
                    ot[:], ps[:], Ident, bias=cb[:, 12 + oc : 13 + oc]
                )
                nc.sync.dma_start(y_d[oc * 128 : (oc + 1) * 128, :], ot[:])

            ps0 = psP.tile([128, S], f32, tag="proj", name="proj")
            out_proj_mm(ps0, 0, range(5), True, False)
            ps1 = psP.tile([128, S], f32, tag="proj", name="proj")
            out_proj_mm(ps1, 1, range(5), True, False)
            norm_bcast(10)
            norm_bcast(11)
            out_proj_mm(ps0, 0, [5], False, True)
            out_proj_fin(ps0, 0)
            out_proj_mm(ps1, 1, [5], False, True)
            out_proj_fin(ps1, 1)
            for oc in range(2, NC):
                ps = psP.tile([128, S], f32, tag="proj", name="proj")
                out_proj_mm(ps, oc, range(NC), True, True)
                out_proj_fin(ps, oc)

    return nc


def _get_nc():
    if "nc" not in _STATE:
        _STATE["nc"] = _build_nc()
    return _STATE["nc"]


# --------------------------------------------------------------------------
def _prep_maps(inputs):
    import ml_dtypes

    bf16 = ml_dtypes.bfloat16
    hs = np.asarray(inputs["hidden_states"], dtype=np.float32)
    Wq = np.asarray(inputs["Wq"], dtype=np.float32)
    bq = np.asarray(inputs["bq"], dtype=np.float32)
    Wk = np.asarray(inputs["Wk"], dtype=np.float32)
    bk = np.asarray(inputs["bk"], dtype=np.float32)
    Wv = np.asarray(inputs["Wv"], dtype=np.float32)
    bv = np.asarray(inputs["bv"], dtype=np.float32)
    Wo = np.asarray(inputs["Wo"], dtype=np.float32)

    # head-major channel permutation: c' = h*64 + d  <-  c = d*12 + h
    idx = (np.arange(H)[:, None] + np.arange(D)[None, :] * H).reshape(C)
    scale = float(D) ** -0.5

    def pack_blocks(wt):
        # wt: [c_in, c_out] -> [oc, p, cc*128 + co]
        w4 = wt.reshape(NC, 128, NC, 128).transpose(2, 1, 0, 3)
        return np.ascontiguousarray(w4.reshape(NC, 128, C)).astype(bf16)

    wqp = pack_blocks((scale * Wq[idx, :]).T)
    wkp = pack_blocks(Wk[idx, :].T)
    wop = pack_blocks(Wo.T)
    # wv: [p, cc*768 + co]
    wvp = np.ascontiguousarray(
        Wv[idx, :].T.reshape(NC, 128, C).transpose(1, 0, 2).reshape(128, NC * C)
    ).astype(bf16)

    cbm = np.zeros((128, 146), dtype=np.float32)
    cbm[:, 0:6] = (scale * bq[idx]).reshape(6, 128).T
    cbm[:, 6:12] = bk[idx].reshape(6, 128).T
    # V-bias folded through attention (softmax rows sum to 1):
    # attn' = attn_nobias' + bv[idx], so out += Wo @ bv[idx]
    cbm[:, 12:18] = (Wo @ bv[idx]).reshape(6, 128).T
    # mask[k, q] = NEG where k < q within a diagonal 128-block
    cbm[:, 18:146] = np.triu(np.full((128, 128), NEG, dtype=np.float32), 1)

    shared = {"wqp": wqp, "wkp": wkp, "wop": wop, "wvp": wvp, "cb": cbm}
    maps = []
    for b in range(B):
        xb = hs[b, :, 0, :].reshape(NC, 128, S).transpose(1, 0, 2)
        xp = np.ascontiguousarray(xb.reshape(128, NC * S)).astype(bf16)
        maps.append({"xp": xp, **shared})
    return maps


def _run(inputs, trace=False, **kwargs):
    from concourse.bass_utils import run_bass_kernel_spmd

    nc = _get_nc()
    in_maps = _prep_maps(inputs)
    res = run_bass_kernel_spmd(
        nc, in_maps, core_ids=list(range(B)), trace=trace, **kwargs
    )
    out = np.stack([res.results[b]["y"] for b in range(B)], axis=0)
    return out.reshape(B, C, 1, S).astype(np.float32), res


def kernel(**inputs):
    out, _ = _run(inputs, trace=False)
    return out


# revision 32
# speedup vs baseline: 1.3628x; 1.0243x over previous
"""Trainium2 Bass kernel for nn_Attention_30468497997979.

Reference computation (per batch b of 8):
    X = hidden_states[b,:,0,:]              # (C=768, S=384)
    Q/K/V = W @ X + b                       # 1x1 conv == channel matmul
    per head h (12 heads, head dim 64, channel c = d*12 + h):
        scores = (Q_h^T K_h) / 8, mask q>k, softmax over k
        attn_h = V_h @ softmax
    out = Wo @ concat_heads(attn)           # channel c = h*64 + d
Sharding: pure data-parallel, one batch per NeuronCore (8 cores).

Per-core kernel design (v2 — DMA/schedule-optimized):
  - All matmul data is bf16 (1 PE col/cycle); PSUM accumulation fp32.
  - Host pre-permutes W_{q,k,v} rows to head-major channel order
    (c' = h*64 + d), transposes to [c_in, c_out], folds 1/sqrt(d) into
    Wq/bq, folds the V bias through attention (softmax rows sum to 1)
    into an output bias Wo @ bv.
  - Weights are host-packed so every SBUF tile is one contiguous
    hardware-DMA: wq/wk/wo as six per-output-block [128, 6*128] loads
    (the kernel consumes output blocks one at a time, so each block
    gets its own completion semaphore), wv/x as [p][chunk][col] packs.
  - DMA priority: x split across three queues first, then wq0/wk0,
    then wv, then remaining wq/wk blocks, wo last — compute starts
    ~3us in while the rest of the weights stream behind it.
  - scores are computed transposed ([k, q], keys on partitions) into
    two PSUM banks per head (k-chunks 0+2 packed into one 512-col
    bank) -> 2 exps per head instead of 3. No max-subtraction needed
    (scores are O(1); masked entries get -1e4 -> exp == 0).
  - attn@V contracts over k on partitions with a fused ones-column in
    each V tile producing the softmax denominator as PSUM row 64; one
    [65, 384] copy moves attn+denominator to SBUF together.
  - Denominators DMA-scatter to an [8, 48] block per head so the DVE
    reciprocal runs on 32 full partitions per 4-head group (~0.4us
    instead of 2.5us on 12 lanes), then a K=1 PE matmul broadcasts
    each row across 64 partitions for the normalize multiply.
  - Q/K projections interleave with attention per 2-head chunk so the
    PE never waits on a cold weight block.
"""

import numpy as np

B, C, S, H, D = 8, 768, 384, 12, 64
NC = C // 128  # 6
NEG = -10000.0

_STATE = {}


# --------------------------------------------------------------------------
# Workaround: this walrus build rejects the multi-wait InstDrain that
# TileContext emits at exit ("Too many sync wait commands"). Split the
# drain's sem waits onto standalone sync-engine wait instructions.
def _patch_tile_drain():
    import concourse.tile as tile_mod
    from concourse.vector_clock import ScopedClock
    from bass_rust import SyncInfo

    if getattr(tile_mod.TileContext, "_drain_split_patch", False):
        return

    def _drain_and_barrier_split(self, tick_clock, wait_clock):
        nc = self.nc
        assert self.sems is not None
        handles = {}
        for h in self.sems.allocated().values():
            handles[h.num] = h
            handles[h.name] = h

        probe = nc.sync.nop()
        wait_clock.add_sem_waits(
            probe.ins, ScopedClock({None: tick_clock.global_clock})
        )
        waits = list(probe.ins.sync_info.on_wait)
        probe.ins.sync_info = SyncInfo(on_wait=[], on_update=[])
        for w in waits:
            h = handles.get(w.id) or handles.get(w.ant_name)
            if h is not None:
                nc.sync.wait_ge(h, w.wait_value)
            else:
                n2 = nc.sync.nop()
                n2.ins.sync_info = SyncInfo(on_wait=[w], on_update=[])

        drain_inst = nc.sync.drain()
        wait_clock.add_sem_waits(
            drain_inst.ins, ScopedClock({None: tick_clock.global_clock})
        )
        if list(drain_inst.ins.sync_info.on_wait):
            drain_inst.ins.sync_info = SyncInfo(on_wait=[], on_update=[])

        nc.all_engine_barrier()
        popped = nc._tile_sem_poison_stack.pop()
        assert popped is self._sem_poison
        nc.clear_and_free_semaphores(list(self.sems.allocated().values()))
        nc.all_engine_barrier()

        # This walrus codegen supports at most ONE sem wait per
        # instruction. Move extra waits onto same-engine nop carriers
        # inserted just before the instruction (engine queues execute in
        # order, so the semantics are identical).
        import concourse.mybir as mybir

        k = 0
        for f in nc.m.functions:
            for bb in f.blocks:
                new_insts = []
                for inst in bb.instructions:
                    si = inst.sync_info
                    waits = list(si.on_wait) if si else []
                    if len(waits) > 1:
                        for w in waits[:-1]:
                            nop = mybir.InstNoOp(name=f"I-wsplit-{k}")
                            k += 1
                            nop.engine = inst.engine
                            nop.sync_info = SyncInfo(on_wait=[w], on_update=[])
                            nc.register_instruction(nop)
                            new_insts.append(nop)
                        inst.sync_info = SyncInfo(
                            on_wait=[waits[-1]], on_update=list(si.on_update)
                        )
                    new_insts.append(inst)
                bb.instructions = new_insts

    tile_mod.TileContext._drain_and_barrier = _drain_and_barrier_split
    tile_mod.TileContext._drain_split_patch = True


# --------------------------------------------------------------------------
def _build_nc():
    import concourse.bass as bass
    import concourse.mybir as mybir
    import concourse.tile as tile

    _patch_tile_drain()

    f32 = mybir.dt.float32
    f32r = mybir.dt.float32r
    bf16 = mybir.dt.bfloat16
    Ident = mybir.ActivationFunctionType.Identity
    Exp = mybir.ActivationFunctionType.Exp

    nc = bass.Bass()
    # host-packed inputs (see _prep_maps for the exact layouts)
    x_d = nc.dram_tensor("xp", [128, NC * S], bf16, kind="ExternalInput")
    wq_d = nc.dram_tensor("wqp", [NC, 128, C], bf16, kind="ExternalInput")
    wk_d = nc.dram_tensor("wkp", [NC, 128, C], bf16, kind="ExternalInput")
    wo_d = nc.dram_tensor("wop", [NC, 128, C], bf16, kind="ExternalInput")
    wv_d = nc.dram_tensor("wvp", [128, NC * C], bf16, kind="ExternalInput")
    # consts: cols 0:6 bq, 6:12 bk, 12:18 obias
    cb_d = nc.dram_tensor("cb", [128, 18], f32, kind="ExternalInput")
    # 0/1 bf16 causal mask block: mb[k, q] = 1 where k >= q
    mb_d = nc.dram_tensor("mb", [128, 128], bf16, kind="ExternalInput")
    y_d = nc.dram_tensor("y", [C, S], f32, kind="ExternalOutput")

    with tile.TileContext(nc) as tc:
        with (
            tc.tile_pool(name="persist", bufs=1) as persist,
            tc.tile_pool(name="epool", bufs=6) as epool,
            tc.tile_pool(name="opool", bufs=3) as opool,
            tc.tile_pool(name="psP", bufs=2, space="PSUM") as psP,
            tc.tile_pool(name="psS", bufs=3, space="PSUM") as psS,
            tc.tile_pool(name="psV", bufs=2, space="PSUM") as psV,
            tc.tile_pool(name="psR", bufs=1, space="PSUM") as psR,
        ):
            # ---- persistent tiles -------------------------------------
            xt = persist.tile([128, NC, S], bf16, tag="x", name="x")
            wvb = persist.tile([128, NC, C], bf16, tag="wv", name="wv")
            wqb = [
                persist.tile([128, NC, 128], bf16, tag=f"wq{i}", name=f"wq{i}")
                for i in range(NC)
            ]
            wkb = [
                persist.tile([128, NC, 128], bf16, tag=f"wk{i}", name=f"wk{i}")
                for i in range(NC)
            ]
            wob = [
                persist.tile([128, NC, 128], bf16, tag=f"wo{i}", name=f"wo{i}")
                for i in range(NC)
            ]
            cb = persist.tile([128, 18], f32, tag="cb", name="cb")
            mb = persist.tile([128, 128], bf16, tag="mb", name="mb")
            q_sb = [
                persist.tile([128, S], bf16, tag=f"q{i}", name=f"q{i}")
                for i in range(NC)
            ]
            k_sb = [
                persist.tile([128, S], bf16, tag=f"k{i}", name=f"k{i}")
                for i in range(NC)
            ]
            vt = [
                persist.tile([128, H, D + 1], bf16, tag=f"vt{sq}", name=f"vt{sq}")
                for sq in range(3)
            ]
            au = [
                persist.tile([D + 1, S], f32, tag=f"au{h}", name=f"au{h}")
                for h in range(H)
            ]
            attn_sb = [
                persist.tile([128, S], bf16, tag=f"at{i}", name=f"at{i}")
                for i in range(NC)
            ]
            # head h's denominator row lives at partitions
            # base(h) .. base(h)+8, 48 q-values per partition
            sums_sb = persist.tile([128, 48], f32, tag="sums", name="sums")
            # 1/sum in bf16: keeps the broadcast matmul all-bf16 (a
            # f32r matmul here forces a PE pipeline mode switch costing
            # ~0.5-1.2us per normalize broadcast)
            rinv_sb = persist.tile([128, 48], bf16, tag="rinv", name="rinv")
            rinv_r = persist.tile([1, H, S], bf16, tag="rinvr", name="rinvr")
            ones_sb = persist.tile([1, D], bf16, tag="ones", name="ones")

            # ---- DMA issue --------------------------------------------
            # Only the Sync and Activation queues are hardware-dynamic
            # (~134 GB/s each); the GpSimd queue is software-dynamic at
            # ~13 GB/s aggregate, so it only carries the tiny SBUF->SBUF
            # denominator moves. Per-queue order = priority (need order).
            # The Activation engine also runs the proj-copy/exp pipeline,
            # so only its startup-critical DMAs are issued up front; the
            # rest are issued from mid-program points (the queue keeps
            # streaming earlier transfers meanwhile).
            nc.scalar.dma_start(xt[:, 0:3, :], x_d[:, 0 : 3 * S])
            nc.scalar.dma_start(wqb[0][:], wq_d[0])
            nc.scalar.dma_start(cb[:], cb_d[:, :])
            nc.scalar.dma_start(mb[:], mb_d[:, :])
            nc.scalar.dma_start(wqb[1][:], wq_d[1])
            nc.sync.dma_start(xt[:, 3:6, :], x_d[:, 3 * S : 6 * S])
            nc.sync.dma_start(wkb[0][:], wk_d[0])
            nc.sync.dma_start(wkb[1][:], wk_d[1])
            nc.sync.dma_start(wvb[:, 3:6, :], wv_d[:, 3 * C : 6 * C])
            for i in range(2, NC):
                nc.sync.dma_start(wkb[i][:], wk_d[i])
            for i in range(NC):
                nc.sync.dma_start(wob[i][:], wo_d[i])

            nc.vector.memset(ones_sb[:], 1.0)
            for sq in range(3):
                nc.vector.memset(vt[sq][:, :, D : D + 1], 1.0)

            # ---- building blocks --------------------------------------
            def qk_proj(oc, wtiles, bias_col, out):
                # copy+bias on DVE (tensor_scalar add) — the Activation
                # engine's FIFO is reserved for the exp chain
                ps = psP.tile([128, S], f32, tag="proj", name="proj")
                for cc in range(NC):
                    nc.tensor.matmul(
                        ps[:],
                        wtiles[oc][:, cc, :],
                        xt[:, cc, :],
                        start=(cc == 0),
                        stop=(cc == NC - 1),
                    )
                nc.vector.tensor_scalar_add(
                    out[:], ps[:], cb[:, bias_col : bias_col + 1]
                )

            def v_proj(half):
                # vt[sq][:, half*6:(half+1)*6, 0:64] = (X^T Wv')[s, c' half]
                # cc order follows wv chunk DMA arrival (sync half 3:6
                # lands first, then the deferred scalar half 0:3)
                cc_order = [3, 4, 5, 0, 1, 2]
                for sq in range(3):
                    ps = psP.tile([128, S], f32, tag="proj", name="proj")
                    for step, cc in enumerate(cc_order):
                        nc.tensor.matmul(
                            ps[:],
                            xt[:, cc, sq * 128 : (sq + 1) * 128],
                            wvb[:, cc, half * 384 : (half + 1) * 384],
                            start=(step == 0),
                            stop=(step == NC - 1),
                        )
                    nc.vector.tensor_copy(
                        vt[sq][:, half * 6 : (half + 1) * 6, 0:D],
                        ps[:].rearrange("p (h d) -> p h d", d=D),
                    )

            def head_scores(h):
                # A = [kc0 (q 0:128) | kc2 (q 0:384)], B = [kc1 (q 0:256)]
                # B runs first so its exp clears the PSUM ring early.
                # Causal masking happens post-exp as a 0/1 multiply on the
                # SBUF e-tiles (exp(-1e4)==0 == exp(s)*0), which keeps the
                # PSUM->exp chain short and runs on the idle engines.
                oc, prow = h // 2, (h % 2) * D
                Qh = q_sb[oc][prow : prow + D, :]
                Kh = k_sb[oc][prow : prow + D, :]
                ps_b = psS.tile([128, 512], f32, tag="s", name="sb")
                ps_a = psS.tile([128, 512], f32, tag="s", name="sa")
                nc.tensor.matmul(
                    ps_b[:, 0:256], Kh[:, 128:256], Qh[:, 0:256],
                    start=True, stop=True,
                )
                eB = epool.tile([128, 256], bf16, tag="eB", name="eB")
                nc.scalar.activation(eB[:], ps_b[:, 0:256], Exp)
                nc.vector.tensor_mul(eB[:, 128:256], eB[:, 128:256], mb[:])
                nc.tensor.matmul(
                    ps_a[:, 0:128], Kh[:, 0:128], Qh[:, 0:128],
                    start=True, stop=True,
                )
                nc.tensor.matmul(
                    ps_a[:, 128:512], Kh[:, 256:384], Qh[:, 0:384],
                    start=True, stop=True, skip_group_check=True,
                )
                eA = epool.tile([128, 512], bf16, tag="eA", name="eA")
                nc.scalar.activation(eA[:], ps_a[:], Exp)
                nc.gpsimd.tensor_mul(eA[:, 0:128], eA[:, 0:128], mb[:])
                nc.gpsimd.tensor_mul(eA[:, 384:512], eA[:, 384:512], mb[:])
                return eA, eB

            def head_av(h, eA, eB):
                # attn@V with fused denominator column; accumulate widest
                # first so every element's first write carries start
                ps_av = psV.tile([D + 1, S], f32, tag="av", name="av")
                nc.tensor.matmul(
                    ps_av[:, 0:384], vt[2][:, h, :], eA[:, 128:512],
                    start=True, stop=False, skip_group_check=True,
                )
                nc.tensor.matmul(
                    ps_av[:, 0:256], vt[1][:, h, :], eB[:, 0:256],
                    start=False, stop=False, skip_group_check=True,
                )
                nc.tensor.matmul(
                    ps_av[:, 0:128], vt[0][:, h, :], eA[:, 0:128],
                    start=False, stop=True, skip_group_check=True,
                )
                nc.vector.tensor_copy(au[h][:], ps_av[:])
                nc.sync.dma_start(
                    sums_sb[_base(h) : _base(h) + 8, :], au[h][D : D + 1, :]
                )

            # normalize groups: heads (0-3), (4-7), (8-9), (10-11) at
            # partition bases 0/32/64/96 (compute ops need 32-aligned
            # partition starts)
            GRP = [range(0, 4), range(4, 8), range(8, 10), range(10, 12)]

            def _base(h):
                g = h // 4 if h < 8 else 2 + (h - 8) // 2
                i = h - GRP[g][0]
                return 32 * g + 8 * i

            def norm_pre(g):
                # per-group reciprocal on full partitions, then repack
                # each head's row into rinv_r's free dim (f32r view)
                hs = GRP[g]
                p0, np_ = 32 * g, 8 * len(hs)
                with nc.allow_low_precision(
                    reason="1/softmax-denom in bf16: denom is O(1-20), "
                    "bf16 keeps ~0.4% relative error, well within budget"
                ):
                    nc.vector.reciprocal(
                        rinv_sb[p0 : p0 + np_, :], sums_sb[p0 : p0 + np_, :]
                    )
                nc.sync.dma_start(
                    rinv_r[0:1, hs[0] : hs[0] + len(hs), :],
                    rinv_sb[p0 : p0 + np_, :],
                )

            def norm_bcast(h):
                # K=1 matmul broadcasts 1/sum across 64 partitions, then
                # the DVE multiply writes the normalized bf16 attn chunk
                oc, prow = h // 2, (h % 2) * D
                ps_r = psR.tile([D, S], f32, tag="rb", name="rb")
                nc.tensor.matmul(
                    ps_r[:], ones_sb[:], rinv_r[0:1, h, :],
                    start=True, stop=True,
                )
                nc.vector.tensor_mul(
                    attn_sb[oc][prow : prow + D, :], au[h][0:D, :], ps_r[:]
                )

            # ---- schedule ---------------------------------------------
            # Q/K projections and scores run ahead while wv streams in;
            # attn@V starts once the V projection lands. Remaining weight
            # DMAs issue from mid-program so the Activation engine's FIFO
            # stays responsive for the proj-copy/exp pipeline. Normalize
            # broadcasts interleave with the next chunk's matmuls to hide
            # the denominator chain (copy->DMA->recip->DMA) latency.
            es = {}
            qk_proj(0, wqb, 0, q_sb[0])
            qk_proj(0, wkb, 6, k_sb[0])
            es[0] = head_scores(0)
            nc.scalar.dma_start(wqb[2][:], wq_d[2])
            es[1] = head_scores(1)
            nc.scalar.dma_start(wqb[3][:], wq_d[3])
            qk_proj(1, wqb, 1, q_sb[1])
            nc.scalar.dma_start(wvb[:, 0:3, :], wv_d[:, 0 : 3 * C])
            qk_proj(1, wkb, 7, k_sb[1])
            nc.scalar.dma_start(wqb[4][:], wq_d[4])
            es[2] = head_scores(2)
            nc.scalar.dma_start(wqb[5][:], wq_d[5])
            es[3] = head_scores(3)
            qk_proj(2, wqb, 2, q_sb[2])
            qk_proj(2, wkb, 8, k_sb[2])
            es[4] = head_scores(4)
            es[5] = head_scores(5)
            v_proj(0)
            v_proj(1)
            for h in range(4):
                head_av(h, *es.pop(h))
            norm_pre(0)
            head_av(4, *es.pop(4))
            head_av(5, *es.pop(5))
            qk_proj(3, wqb, 3, q_sb[3])
            norm_bcast(0)
            qk_proj(3, wkb, 9, k_sb[3])
            norm_bcast(1)
            es[6] = head_scores(6)
            norm_bcast(2)
            es[7] = head_scores(7)
            norm_bcast(3)
            head_av(6, *es.pop(6))
            head_av(7, *es.pop(7))
            norm_pre(1)
            qk_proj(4, wqb, 4, q_sb[4])
            qk_proj(4, wkb, 10, k_sb[4])
            es[8] = head_scores(8)
            norm_bcast(4)
            es[9] = head_scores(9)
            norm_bcast(5)
            head_av(8, *es.pop(8))
            norm_bcast(6)
            head_av(9, *es.pop(9))
            norm_bcast(7)
            norm_pre(2)
            qk_proj(5, wqb, 5, q_sb[5])
            qk_proj(5, wkb, 11, k_sb[5])
            es[10] = head_scores(10)
            norm_bcast(8)
            es[11] = head_scores(11)
            norm_bcast(9)
            head_av(10, *es.pop(10))
            head_av(11, *es.pop(11))
            norm_pre(3)

            # ---- output projection (bias = host-folded Wo @ bv) -------
            # the first two chunks accumulate cc 0..4 before the last
            # normalize group's broadcasts, hiding that chain's latency;
            # their cc=5 step follows the group-3 multiplies
            def out_proj_mm(ps, oc, ccs, start, stop):
                for i, cc in enumerate(ccs):
                    nc.tensor.matmul(
                        ps[:],
                        wob[oc][:, cc, :],
                        attn_sb[cc][:],
                        start=(start and i == 0),
                        stop=(stop and i == len(ccs) - 1),
                    )

            def out_proj_fin(ps, oc):
                ot = opool.tile([128, S], f32, tag="o", name="o")
                nc.scalar.activation(
                    ot[:], ps[:], Ident, bias=cb[:, 12 + oc : 13 + oc]
                )
                nc.sync.dma_start(y_d[oc * 128 : (oc + 1) * 128, :], ot[:])

            ps0 = psP.tile([128, S], f32, tag="proj", name="proj")
            out_proj_mm(ps0, 0, range(5), True, False)
            ps1 = psP.tile([128, S], f32, tag="proj", name="proj")
            out_proj_mm(ps1, 1, range(5), True, False)
            norm_bcast(10)
            norm_bcast(11)
            out_proj_mm(ps0, 0, [5], False, True)
            out_proj_fin(ps0, 0)
            out_proj_mm(ps1, 1, [5], False, True)
            out_proj_fin(ps1, 1)
            for oc in range(2, NC):
                ps = psP.tile([128, S], f32, tag="proj", name="proj")
                out_proj_mm(ps, oc, range(NC), True, True)
                out_proj_fin(ps, oc)

    return nc


def _get_nc():
    if "nc" not in _STATE:
        _STATE["nc"] = _build_nc()
    return _STATE["nc"]


# --------------------------------------------------------------------------
def _prep_maps(inputs):
    import ml_dtypes

    bf16 = ml_dtypes.bfloat16
    hs = np.asarray(inputs["hidden_states"], dtype=np.float32)
    Wq = np.asarray(inputs["Wq"], dtype=np.float32)
    bq = np.asarray(inputs["bq"], dtype=np.float32)
    Wk = np.asarray(inputs["Wk"], dtype=np.float32)
    bk = np.asarray(inputs["bk"], dtype=np.float32)
    Wv = np.asarray(inputs["Wv"], dtype=np.float32)
    bv = np.asarray(inputs["bv"], dtype=np.float32)
    Wo = np.asarray(inputs["Wo"], dtype=np.float32)

    # head-major channel permutation: c' = h*64 + d  <-  c = d*12 + h
    idx = (np.arange(H)[:, None] + np.arange(D)[None, :] * H).reshape(C)
    scale = float(D) ** -0.5

    def pack_blocks(wt):
        # wt: [c_in, c_out] -> [oc, p, cc*128 + co]
        w4 = wt.reshape(NC, 128, NC, 128).transpose(2, 1, 0, 3)
        return np.ascontiguousarray(w4.reshape(NC, 128, C)).astype(bf16)

    wqp = pack_blocks((scale * Wq[idx, :]).T)
    wkp = pack_blocks(Wk[idx, :].T)
    wop = pack_blocks(Wo.T)
    # wv: [p, cc*768 + co]
    wvp = np.ascontiguousarray(
        Wv[idx, :].T.reshape(NC, 128, C).transpose(1, 0, 2).reshape(128, NC * C)
    ).astype(bf16)

    cbm = np.zeros((128, 18), dtype=np.float32)
    cbm[:, 0:6] = (scale * bq[idx]).reshape(6, 128).T
    cbm[:, 6:12] = bk[idx].reshape(6, 128).T
    # V-bias folded through attention (softmax rows sum to 1):
    # attn' = attn_nobias' + bv[idx], so out += Wo @ bv[idx]
    cbm[:, 12:18] = (Wo @ bv[idx]).reshape(6, 128).T
    # post-exp causal mask for a diagonal 128-block: keep k >= q
    mbm = np.tril(np.ones((128, 128), dtype=np.float32)).astype(bf16)

    shared = {"wqp": wqp, "wkp": wkp, "wop": wop, "wvp": wvp,
              "cb": cbm, "mb": mbm}
    maps = []
    for b in range(B):
        xb = hs[b, :, 0, :].reshape(NC, 128, S).transpose(1, 0, 2)
        xp = np.ascontiguousarray(xb.reshape(128, NC * S)).astype(bf16)
        maps.append({"xp": xp, **shared})
    return maps


def _run(inputs, trace=False, **kwargs):
    from concourse.bass_utils import run_bass_kernel_spmd

    nc = _get_nc()
    in_maps = _prep_maps(inputs)
    res = run_bass_kernel_spmd(
        nc, in_maps, core_ids=list(range(B)), trace=trace, **kwargs
    )
    out = np.stack([res.results[b]["y"] for b in range(B)], axis=0)
    return out.reshape(B, C, 1, S).astype(np.float32), res


def kernel(**inputs):
    out, _ = _run(inputs, trace=False)
    return out
